# revision 1
# baseline (speedup 1.0000x reference)
"""Trainium2 Bass kernel for nn_Encoder_Block (dense transformer encoder block).

Strategy: pure data parallel across 8 NeuronCores (B=16 -> 2 batch elems per
core), all weights replicated.  Entire block computed on-chip per batch elem:
  x + pos -> LN0 (res) -> 4x [dsconv -> relu -> +res -> LN] -> attention
  -> +res -> LNe -> FC -> relu -> +res

Host-side weight folding (constant prep, input-independent layout changes):
  - depthwise(k=7) + pointwise conv folded into 7 full [C,C] matmul taps:
      W2[c,o; tap d] = pw_w[o,c] * dw_w[c,d]    (accumulated in PSUM over d)
  - conv bias folded: b2 = pw_w @ dw_b + pw_b
  - attention softmax scale folded into Wq;  both (identical) heads folded:
      Wo_eff = Wo[:dk] + Wo[dk:]
  - positional-encoding table precomputed (input-independent constant)

Matmuls run as float32r (hardware fast-fp32 mode, 1 cycle/row at N>=256).
LayerNorm over (C,T) jointly: per-partition sums via accum_out / Square-accum,
cross-partition reduction + broadcast via tiny ones-matmuls through PSUM;
rsqrt computed as exp(-0.5*ln(var+eps)) so the whole kernel stays in one
scalar-engine table set (natural_log_exp_and_others).
Softmax: scores computed transposed [tk,tq]; exp without max-subtraction
(values bounded, safe in fp32); denominator via an appended ones-column in
the PV matmul; normalization by reciprocal broadcast through a ones-matmul.
"""

import sys

sys.path.insert(0, "/opt/trn_rl_repo")

import math

import numpy as np

import concourse.bass as bass
import concourse.tile as tile
from concourse import bacc, mybir
from concourse.bass_utils import run_bass_kernel_spmd

F32 = mybir.dt.float32
F32R = mybir.dt.float32r
AF = mybir.ActivationFunctionType
ALU = mybir.AluOpType

B, C, T = 16, 128, 1024
NCONV, KW = 4, 7
DK = C // 2
NCORES = 8
BPC = B // NCORES          # batch elems per core
EPS = 1e-5
NEL = float(C * T)         # layernorm element count
PADT = T + KW - 1          # 1030: per-batch padded row in xpad
NLN = NCONV + 2            # LN0, 4 conv LNs, LNe


def _pos_encoding() -> np.ndarray:
    i = np.arange(C)
    exp = -((i - (i % 2)).astype(np.float32) / np.float32(C))
    freqs = (np.float32(10000.0) ** exp)[:, None].astype(np.float32)
    phases = ((i % 2).astype(np.float32) * np.float32(np.pi / 2))[:, None]
    pos = np.arange(T, dtype=np.float32)[None, :]
    return np.sin(pos * freqs + phases).astype(np.float32)


def _uniform_val(a: np.ndarray):
    """Return the scalar if all elements equal, else None."""
    v = a.flat[0]
    return float(v) if np.all(a == v) else None


class _Flags:
    """Kernel-structure flags derived from host inspection of the inputs."""

    def __init__(self, ln_gc, ln_bc, b2_zero, fcb_zero, mask_ones):
        self.ln_gc = tuple(ln_gc)
        self.ln_bc = tuple(ln_bc)
        self.b2_zero = b2_zero
        self.fcb_zero = fcb_zero
        self.mask_ones = mask_ones

    def key(self):
        return (self.ln_gc, self.ln_bc, self.b2_zero, self.fcb_zero,
                self.mask_ones)


DEBUG_TAPS = False   # set True (by debug harnesses only) to dump intermediates


class _Bacc(bacc.Bacc):
    """Bacc with activation-table choice pinned to the one set that covers
    every function this kernel uses (ln/exp/square/copy/relu/identity).

    The stock insertion pass greedily picks the first act_info set containing
    each function, which lands exp/square in `exp_and_others` and ln in
    `natural_log` — ping-ponging ~2 table loads (~5us) per layernorm.  Hiding
    our functions from every other set (ids/positions unchanged) forces all
    loads to `natural_log_exp_and_others`, so exactly one load is emitted.
    """

    _OURS = {AF.Ln, AF.Exp, AF.Square, AF.Copy, AF.Identity, AF.Relu}
    _KEEP = "natural_log_exp_and_others"

    def insert_act_table_loads(self):
        from concourse.bacc import _bass_rust, get_activation_tables
        has_activation = any(
            isinstance(i, mybir.InstActivation)
            for b in self.main_func.blocks
            for i in b.instructions
        )
        if not has_activation:
            return
        tables = [
            (nm, fs if nm == self._KEEP else (fs - self._OURS))
            for nm, fs in get_activation_tables(self.m.arch).items()
        ]
        _bass_rust.insert_act_table_loads(self, tables)


def _build(flags: _Flags):
    nc = _Bacc("TRN2", target_bir_lowering=False, debug=False,
               num_devices=NCORES)
    def tap(name, ap):
        if DEBUG_TAPS:
            d = nc.dram_tensor("dbg_" + name, list(ap.shape), F32,
                               kind="ExternalOutput").ap()
            nc.sync.dma_start(d, ap if ap.dtype == F32 else ap.bitcast(F32))

    def dram(name, shape, kind="ExternalInput"):
        return nc.dram_tensor(name, shape, F32, kind=kind).ap()

    x_d = dram("x", [BPC, C, T])
    pos_d = dram("pos", [C, T])
    w2_d = dram("w2", [C, NCONV * KW * C])
    wqkv_d = dram("wqkv", [C, 3 * DK])
    wo_d = dram("wo", [DK, C])
    fcw_d = dram("fcw", [C, C])
    out_d = dram("out", [BPC, C, T], kind="ExternalOutput")
    if not flags.b2_zero:
        b2_d = dram("b2", [C, NCONV])
    if not flags.fcb_zero:
        fcb_d = dram("fcb", [C, 1])
    nonuni = [l for l in range(NLN)
              if flags.ln_gc[l] is None or flags.ln_bc[l] is None]
    gb_entries = ([(l, "g") for l in range(NLN) if flags.ln_gc[l] is None]
                  + [(l, "b") for l in range(NLN) if flags.ln_bc[l] is None])
    if gb_entries:
        gb_d = dram("gb", [C, len(gb_entries) * T])
    if not flags.mask_ones:
        mb_d = dram("mb", [C, BPC * (T // C)])   # [128, 2*8] key-mask exp bias
        qm_d = dram("qm", [BPC, T])              # query-mask rows

    from contextlib import ExitStack

    with tile.TileContext(nc) as tc, ExitStack() as ctx:
        cst = ctx.enter_context(tc.tile_pool(name="cst", bufs=1))
        big = ctx.enter_context(tc.tile_pool(name="bigbuf", bufs=1))
        resp = ctx.enter_context(tc.tile_pool(name="resp", bufs=2))
        work = ctx.enter_context(tc.tile_pool(name="work", bufs=2))
        tiny = ctx.enter_context(tc.tile_pool(name="tiny", bufs=4))
        ps_big = ctx.enter_context(
            tc.tile_pool(name="ps_big", bufs=2, space="PSUM"))
        ps_sm = ctx.enter_context(
            tc.tile_pool(name="ps_sm", bufs=2, space="PSUM"))
        ps_av = ctx.enter_context(
            tc.tile_pool(name="ps_av", bufs=1, space="PSUM"))

        # ---- constants / weights in SBUF ----
        ones_col = cst.tile([C, 1], F32, tag="ones_col")
        nc.vector.memset(ones_col[:], 1.0)
        const_tiles: dict = {}

        def const_ap(val: float, npart: int = C):
            """[npart,1] fp32 SBUF constant (activation bias operand)."""
            if val == 0.0:
                return 0.0   # pre-registered const AP
            if val not in const_tiles:
                t = cst.tile([C, 1], F32, tag=f"cst{len(const_tiles)}")
                nc.vector.memset(t[:], val)
                const_tiles[val] = t
            return const_tiles[val][0:npart, :]
        ones_row = cst.tile([1, C], F32, tag="ones_row")
        nc.vector.memset(ones_row[:], 1.0)
        ones8 = cst.tile([C, 8], F32, tag="ones8")
        nc.vector.memset(ones8[:], 1.0)
        ones_row_r = cst.tile([1, C], F32R, tag="ones_row_r")
        nc.scalar.copy(ones_row_r[:], ones_row[:])
        xin = big.tile([C, BPC * T], F32, tag="xin")
        for b in range(BPC):
            nc.sync.dma_start(xin[:, b * T:(b + 1) * T], x_d[b])
        # Prologue loads spread across per-engine HWDGE queues (each
        # engine's dma_start enqueues on its own q<Eng>DynamicHW) so the
        # x/pos loads and the per-layer conv-weight pieces stream in
        # parallel instead of serializing on one queue.
        pos_sb = cst.tile([C, T], F32, tag="pos")
        nc.sync.dma_start(pos_sb[:], pos_d[:])
        w2_sb = cst.tile([C, NCONV * KW * C], F32R, tag="w2")
        LW = KW * C
        nc.sync.dma_start(w2_sb[:, 0:LW], w2_d[:, 0:LW].bitcast(F32R))
        wqkv_sb = cst.tile([C, 3 * DK], F32R, tag="wqkv")
        nc.sync.dma_start(wqkv_sb[:], wqkv_d.bitcast(F32R))
        _w2_eng = [nc.sync, nc.sync, nc.sync]
        for li in range(1, NCONV):
            _w2_eng[li - 1].dma_start(
                w2_sb[:, li * LW:(li + 1) * LW],
                w2_d[:, li * LW:(li + 1) * LW].bitcast(F32R))
        wo_sb = cst.tile([DK, C], F32R, tag="wo")
        nc.sync.dma_start(wo_sb[:], wo_d.bitcast(F32R))
        fcw_sb = cst.tile([C, C], F32R, tag="fcw")
        nc.sync.dma_start(fcw_sb[:], fcw_d.bitcast(F32R))
        if not flags.b2_zero:
            b2_sb = cst.tile([C, NCONV], F32, tag="b2")
            nc.sync.dma_start(b2_sb[:], b2_d[:])
        if not flags.fcb_zero:
            fcb_sb = cst.tile([C, 1], F32, tag="fcb")
            nc.sync.dma_start(fcb_sb[:], fcb_d[:])
        if gb_entries:
            gb_sb = cst.tile([C, len(gb_entries) * T], F32, tag="gb")
            nc.sync.dma_start(gb_sb[:], gb_d[:])
            gb_ix = {e: i for i, e in enumerate(gb_entries)}

            def gb_ap(l, kind):
                i0 = gb_ix[(l, kind)] * T
                return gb_sb[:, i0:i0 + T]
        if not flags.mask_ones:
            mb_sb = cst.tile([C, BPC * (T // C)], F32, tag="mb")
            nc.sync.dma_start(mb_sb[:], mb_d[:])
            qm_sb = cst.tile([1, BPC * T], F32, tag="qm")
            for b in range(BPC):
                nc.sync.dma_start(qm_sb[:, b * T:(b + 1) * T], qm_d[b:b + 1, :])

        # conv/attention input activations, zero-padded per batch elem:
        # [0^3 | x_b (1024) | 0^3] so every conv tap is a full N=512 matmul
        # (f32r requires an even moving-operand size).  Pad zeros written
        # once via ACT (f32r-rounding output keeps the BIR verifier happy);
        # later writes only touch the interior.
        xpad = big.tile([C, BPC * PADT], F32R, tag="xpad")
        for b in range(BPC):
            nc.scalar.mul(xpad[:, b * PADT:b * PADT + 3],
                          ones8[:, 0:3], 0.0)
            nc.scalar.mul(xpad[:, b * PADT + 3 + T:(b + 1) * PADT],
                          ones8[:, 0:3], 0.0)

        sq_scr = big.tile([C, BPC * T], F32, tag="sq_scr")
        out_sb = big.tile([C, BPC * T], F32, tag="out_sb")

        def ip(b, off=0, n=T):
            """AP of the xpad interior for batch b (f32r)."""
            return xpad[:, b * PADT + 3 + off: b * PADT + 3 + off + n]

        def emit_ln_b(ln_idx, b, stats_b, src, dst):
            """LayerNorm over (C,T) for one batch elem (independent chains
            per batch so the two batch pipelines overlap).

            stats_b: [128,2] tile; col 0 holds per-partition sums (S1) from
            the producing op's accum_out; col 1 is written here (S2).
            src: fp32 [C,T] AP; dst: [C,T] AP in its final dtype (f32r when
            the result feeds matmuls).
            """
            nc.scalar.activation(sq_scr[:, b * T:(b + 1) * T], src,
                                 AF.Square, accum_out=stats_b[:, 1:2])
            pss = ps_sm.tile([1, 2], F32, tag="ps_sm")
            nc.tensor.matmul(pss[:], ones_col[:], stats_b[:], start=True,
                             stop=True)
            ssum = tiny.tile([1, 2], F32, tag="ssum")
            nc.scalar.copy(ssum[:], pss[:])
            psb = ps_sm.tile([C, 2], F32, tag="ps_sm")
            nc.tensor.matmul(psb[:], ones_row[:], ssum[:], start=True,
                             stop=True)
            mom = tiny.tile([C, 2], F32, tag="mom")     # [mu, m2]
            nc.vector.tensor_scalar_mul(mom[:], psb[:], 1.0 / NEL)
            mu = mom[:, 0:1]
            # nvar = mu*mu - m2 = -(var); folded back by Ln(scale=-1)
            nvar = tiny.tile([C, 1], F32, tag="nvar")
            nc.vector.scalar_tensor_tensor(nvar[:], mu, mu, mom[:, 1:2],
                                           op0=ALU.mult, op1=ALU.subtract)
            # rs = gc / sqrt(var + eps) = exp(-0.5*ln(var+eps) + ln(gc));
            # stays in the natural_log_exp_and_others ACT table set.
            gc, bc = flags.ln_gc[ln_idx], flags.ln_bc[ln_idx]
            lnv = tiny.tile([C, 1], F32, tag="lnv")
            nc.scalar.activation(lnv[:], nvar[:], AF.Ln, scale=-1.0,
                                 bias=const_ap(EPS))
            rs = tiny.tile([C, 1], F32, tag="rs")
            expb = math.log(gc) if (gc is not None and gc > 0.0) else 0.0
            nc.scalar.activation(rs[:], lnv[:], AF.Exp, scale=-0.5,
                                 bias=const_ap(expb))
            if gc is not None and gc <= 0.0 and gc != 1.0:
                rs2 = tiny.tile([C, 1], F32, tag="rs2")
                nc.vector.tensor_scalar_mul(rs2[:], rs[:], gc)
                rs = rs2
            # op list: normalize, then per-element / uniform g and b as
            # needed.  Only the FINAL op writes dst (whose dtype may be f32r
            # feeding matmuls; the BIR verifier requires every producer of
            # such a tensor to round, so intermediates go through fp32
            # scratch regions that are dead at this point).
            post = []
            if gc is None:
                post.append(lambda i, o: nc.vector.tensor_tensor(
                    o, i, gb_ap(ln_idx, "g"), ALU.mult))
            if bc is None:
                post.append(lambda i, o: nc.vector.tensor_tensor(
                    o, i, gb_ap(ln_idx, "b"), ALU.add))
            elif bc != 0.0:
                post.append(lambda i, o: nc.vector.tensor_scalar_add(
                    o, i, bc))
            if not post:
                # chunked so a following conv's first-half matmuls (reading
                # interior cols <= 515) start before the tail is written
                nc.vector.tensor_scalar(dst[:, 0:520], src[:, 0:520], mu,
                                        rs[:], op0=ALU.subtract, op1=ALU.mult)
                nc.vector.tensor_scalar(dst[:, 520:T], src[:, 520:T], mu,
                                        rs[:], op0=ALU.subtract, op1=ALU.mult)
            else:
                mids = [sq_scr[:, b * T:(b + 1) * T],
                        out_sb[:, b * T:(b + 1) * T]]
                nc.vector.tensor_scalar(mids[0], src, mu, rs[:],
                                        op0=ALU.subtract, op1=ALU.mult)
                for i, emit in enumerate(post):
                    last = (i == len(post) - 1)
                    emit(mids[i % 2], dst if last else mids[(i + 1) % 2])

        # ---- x + pos -> xpad (conv0 input); LN0 -> res ----
        res = resp.tile([C, BPC * T], F32, tag="res")
        for b in range(BPC):
            stats_b = tiny.tile([C, 2], F32, tag="stats")
            nc.vector.scalar_tensor_tensor(
                ip(b), xin[:, b * T:(b + 1) * T], 1.0, pos_sb[:],
                op0=ALU.mult, op1=ALU.add, accum_out=stats_b[:, 0:1])
            if b == 0:
                tap("out0", ip(0))
            emit_ln_b(0, b, stats_b, ip(b).bitcast(F32),
                      res[:, b * T:(b + 1) * T])
        tap("res0", res[:, 0:T])

        # ---- conv layers ----
        for li in range(NCONV):
            new_res = resp.tile([C, BPC * T], F32, tag="res")
            for b in range(BPC):
                stats_b = tiny.tile([C, 2], F32, tag="stats")
                psc = ps_big.tile([C, T], F32, tag="ps_main")
                for h in range(2):
                    for d in range(KW):
                        nc.tensor.matmul(
                            psc[:, h * 512:(h + 1) * 512],
                            w2_sb[:, (li * KW + d) * C:(li * KW + d + 1) * C],
                            xpad[:, b * PADT + h * 512 + d:
                                 b * PADT + h * 512 + d + 512],
                            start=(d == 0), stop=(d == KW - 1))
                nr = new_res[:, b * T:(b + 1) * T]
                if flags.b2_zero:
                    nc.vector.scalar_tensor_tensor(
                        nr, psc[:], 0.0, res[:, b * T:(b + 1) * T],
                        op0=ALU.max, op1=ALU.add,
                        accum_out=stats_b[:, 0:1])
                else:
                    relu_t = work.tile([C, T], F32, tag="relu_t")
                    nc.scalar.activation(relu_t[:], psc[:], AF.Relu,
                                         bias=b2_sb[:, li:li + 1])
                    nc.vector.scalar_tensor_tensor(
                        nr, relu_t[:], 1.0, res[:, b * T:(b + 1) * T],
                        op0=ALU.mult, op1=ALU.add,
                        accum_out=stats_b[:, 0:1])
                if b == 0:
                    tap(f"out{li + 1}", new_res[:, 0:T])
                emit_ln_b(1 + li, b, stats_b, nr, ip(b))
                if b == 0:
                    tap(f"xn{li}", ip(0))
            res = new_res

        # ---- attention (input: normalized, in xpad interior) ----
        new_res = resp.tile([C, BPC * T], F32, tag="res")
        for b in range(BPC):
            stats_b = tiny.tile([C, 2], F32, tag="stats")
            xa = ip(b)  # [128, 1024] f32r
            # q (pre-scaled by 1/sqrt(dk)) and k, side by side on parts 0:64
            psq = ps_av.tile([DK + 1, T], F32, tag="ps_av")
            psk = ps_big.tile([C, T], F32, tag="ps_main")
            for h in range(2):
                nc.tensor.matmul(psq[0:DK, h * 512:(h + 1) * 512],
                                 wqkv_sb[:, 0:DK], xa[:, h * 512:h * 512 + 512],
                                 start=True, stop=True)
                nc.tensor.matmul(psk[0:DK, h * 512:(h + 1) * 512],
                                 wqkv_sb[:, DK:2 * DK],
                                 xa[:, h * 512:h * 512 + 512],
                                 start=True, stop=True)
            qk = work.tile([DK, 2 * T], F32R, tag="qk")
            # halves split so score matmuls start after the first halves;
            # q on DVE, k on ACT so the copies run in parallel
            nc.vector.tensor_copy(qk[:, 0:512], psq[0:DK, 0:512])
            nc.scalar.copy(qk[:, T:T + 512], psk[0:DK, 0:512])
            nc.vector.tensor_copy(qk[:, 512:T], psq[0:DK, 512:T])
            nc.scalar.copy(qk[:, T + 512:2 * T], psk[0:DK, 512:T])
            if b == 0:
                tap("qk", qk[:])
            # v in [t, d] layout: lhsT = x chunks
            psv = ps_sm.tile([C, 512], F32, tag="ps_sm")
            for j in range(8):
                nc.tensor.matmul(psv[:, j * DK:(j + 1) * DK],
                                 xa[:, j * C:(j + 1) * C],
                                 wqkv_sb[:, 2 * DK:3 * DK],
                                 start=True, stop=True)
            vt = work.tile([C, 8, DK + 1], F32R, tag="vt")
            nc.scalar.copy(vt[:, :, DK:DK + 1],
                           ones8[:].rearrange("p (j o) -> p j o", o=1))
            nc.vector.tensor_copy(
                vt[:, :, 0:DK],
                psv[:].rearrange("p (j k) -> p j k", k=DK))
            # scores (transposed: [tk, tq]) + exp, per key chunk
            # single-buffer eT when per-element g/b tables occupy SBUF
            eT = work.tile([C, 8 * T], F32R, tag="eT",
                           bufs=(1 if nonuni else 2))
            for j in range(8):
                pss = ps_big.tile([C, T], F32, tag="ps_main")
                for h in range(2):
                    nc.tensor.matmul(pss[:, h * 512:(h + 1) * 512],
                                     qk[:, T + j * C:T + (j + 1) * C],
                                     qk[:, h * 512:h * 512 + 512],
                                     start=True, stop=True)
                if flags.mask_ones:
                    nc.scalar.activation(eT[:, j * T:(j + 1) * T], pss[:],
                                         AF.Exp)
                else:
                    nc.scalar.activation(eT[:, j * T:(j + 1) * T], pss[:],
                                         AF.Exp,
                                         bias=mb_sb[:, b * 8 + j:b * 8 + j + 1])
            # attention-weighted values + denominator (appended ones column)
            psa = ps_av.tile([DK + 1, T], F32, tag="ps_av")
            for h in range(2):
                for j in range(8):
                    nc.tensor.matmul(
                        psa[:, h * 512:(h + 1) * 512], vt[:, j, :],
                        eT[:, j * T + h * 512: j * T + h * 512 + 512],
                        start=(j == 0), stop=(j == 7))
            if b == 0:
                tap("eT", eT[:])
                tap("vt", vt[:])
            # Per-half tail pipeline: the h=0 denominator/normalize/Wo runs
            # while the h=1 PV matmuls are still accumulating.
            # rr = 1/denom as exp(-ln(denom)): stays in the ln/exp ACT table
            # set (reciprocal_approx_fast miscomputes on this HW path, and
            # DVE reciprocal is 8 cycles/elem).  Reads the denominator row
            # straight from PSUM, in parallel with the av copy.  The +1e-30
            # bias guards fully-masked queries (denom==0 -> huge rr, but av
            # rows are 0).
            av = work.tile([DK + 1, T], F32, tag="av")
            rr = tiny.tile([1, T], F32R, tag="rr", bufs=2)
            lnd = tiny.tile([1, T], F32, tag="lnd", bufs=2)
            avn = work.tile([DK, T], F32R, tag="avn")
            psr = ps_big.tile([C, T], F32, tag="ps_main")
            pso = ps_big.tile([C, T], F32, tag="ps_main")
            for h in range(2):
                sl = slice(h * 512, (h + 1) * 512)
                nc.vector.tensor_copy(av[0:DK, sl], psa[0:DK, sl])
                nc.scalar.activation(lnd[:, sl], psa[DK:DK + 1, sl], AF.Ln,
                                     bias=(0.0 if flags.mask_ones
                                           else const_ap(1e-30, 1)))
                nc.scalar.activation(rr[:, sl], lnd[:, sl], AF.Exp,
                                     scale=-1.0)
                nc.tensor.matmul(psr[0:DK, sl], ones_row_r[:, 0:DK],
                                 rr[:, sl], start=True, stop=True)
                nc.vector.tensor_tensor(avn[:, sl], av[0:DK, sl],
                                        psr[0:DK, sl], ALU.mult)
                nc.tensor.matmul(pso[:, sl], wo_sb[:], avn[:, sl],
                                 start=True, stop=True)
            if b == 0:
                tap("rr", rr[:])
                tap("avn", avn[:])
            nr = new_res[:, b * T:(b + 1) * T]
            if flags.mask_ones:
                nc.vector.scalar_tensor_tensor(
                    nr, pso[:], 1.0, res[:, b * T:(b + 1) * T],
                    op0=ALU.mult, op1=ALU.add, accum_out=stats_b[:, 0:1])
            else:
                # query-mask rows broadcast across partitions (via ones
                # matmul), staged through SBUF (DVE can read only one PSUM
                # operand per instruction)
                qmb = work.tile([C, T], F32, tag="qmb")
                att = work.tile([C, T], F32, tag="att_m")
                for h in range(2):
                    psm = ps_sm.tile([C, 512], F32, tag="ps_sm")
                    nc.tensor.matmul(psm[:], ones_row[:],
                                     qm_sb[:, b * T + h * 512:
                                           b * T + (h + 1) * 512],
                                     start=True, stop=True)
                    nc.scalar.copy(qmb[:, h * 512:(h + 1) * 512], psm[:])
                nc.vector.tensor_tensor(att[:], pso[:], qmb[:], ALU.mult)
                nc.vector.scalar_tensor_tensor(
                    nr, att[:], 1.0, res[:, b * T:(b + 1) * T],
                    op0=ALU.mult, op1=ALU.add, accum_out=stats_b[:, 0:1])
            if b == 0:
                tap("out_attn", new_res[:, 0:T])
            emit_ln_b(NLN - 1, b, stats_b, nr, ip(b))
            if b == 0:
                tap("xne", ip(0))
        res = new_res

        # ---- final FC + relu + residual ----
        for b in range(BPC):
            psf = ps_big.tile([C, T], F32, tag="ps_main")
            for h in range(2):
                nc.tensor.matmul(psf[:, h * 512:(h + 1) * 512], fcw_sb[:],
                                 ip(b, h * 512, 512), start=True, stop=True)
            for h in range(2):
                sl = slice(h * 512, (h + 1) * 512)
                ob = out_sb[:, b * T + h * 512:b * T + (h + 1) * 512]
                rb_ = res[:, b * T + h * 512:b * T + (h + 1) * 512]
                if flags.fcb_zero:
                    nc.vector.scalar_tensor_tensor(
                        ob, psf[:, sl], 0.0, rb_, op0=ALU.max, op1=ALU.add)
                else:
                    relu_t = work.tile([C, T], F32, tag="relu_t")
                    nc.scalar.activation(relu_t[:, sl], psf[:, sl], AF.Relu,
                                         bias=fcb_sb[:])
                    nc.vector.tensor_tensor(ob, relu_t[:, sl], rb_, ALU.add)
                nc.sync.dma_start(out_d[b][:, sl], ob)

    nc.compile()
    return nc


_CACHE: dict = {}
LAST_RUN: dict = {}   # exposed for test harnesses (nc, in_maps)


def kernel(x, mask, dw_w, dw_b, pw_w, pw_b, norm0_g, norm0_b,
           norms_g, norms_b, norme_g, norme_b,
           Wq, Wk, Wv, Wo, fc_w, fc_b):
    x = np.asarray(x, dtype=np.float32)
    mask = np.asarray(mask, dtype=np.float32)

    # ---- host-side constant folding ----
    w2 = np.empty((C, NCONV, KW, C), dtype=np.float32)
    for i in range(NCONV):
        pwT = np.asarray(pw_w[i], np.float32).T          # [c, o]
        for d in range(KW):
            w2[:, i, d, :] = pwT * np.asarray(dw_w[i][:, d],
                                              np.float32)[:, None]
    w2 = w2.reshape(C, NCONV * KW * C)
    b2 = np.stack([np.asarray(pw_w[i], np.float32)
                   @ np.asarray(dw_b[i], np.float32)
                   + np.asarray(pw_b[i], np.float32)
                   for i in range(NCONV)], axis=1)        # [C, NCONV]
    wqkv = np.concatenate([np.asarray(Wq, np.float32) / math.sqrt(DK),
                           np.asarray(Wk, np.float32),
                           np.asarray(Wv, np.float32)], axis=1)  # [C, 3*DK]
    wo = np.asarray(Wo, np.float32)
    wo_eff = np.ascontiguousarray(wo[:DK] + wo[DK:])      # [DK, C]
    fcw = np.ascontiguousarray(np.asarray(fc_w, np.float32).T)  # [c, o]
    fcb = np.asarray(fc_b, np.float32).reshape(C, 1)
    pos = _pos_encoding()

    gs = [norm0_g] + [norms_g[i] for i in range(NCONV)] + [norme_g]
    bs = [norm0_b] + [norms_b[i] for i in range(NCONV)] + [norme_b]
    ln_gc = [_uniform_val(np.asarray(g, np.float32)) for g in gs]
    ln_bc = [_uniform_val(np.asarray(bb, np.float32)) for bb in bs]
    flags = _Flags(ln_gc, ln_bc,
                   b2_zero=not b2.any(),
                   fcb_zero=not fcb.any(),
                   mask_ones=bool(np.all(mask == 1.0)))

    key = flags.key()
    if key not in _CACHE:
        _CACHE[key] = _build(flags)
    nc = _CACHE[key]

    base = {"pos": pos, "w2": w2, "wqkv": wqkv, "wo": wo_eff, "fcw": fcw}
    if not flags.b2_zero:
        base["b2"] = np.ascontiguousarray(b2)
    if not flags.fcb_zero:
        base["fcb"] = np.ascontiguousarray(fcb)
    gb_entries = ([(l, "g") for l in range(NLN) if flags.ln_gc[l] is None]
                  + [(l, "b") for l in range(NLN) if flags.ln_bc[l] is None])
    if gb_entries:
        gb = np.empty((C, len(gb_entries) * T), np.float32)
        for i, (l, kind) in enumerate(gb_entries):
            src = gs[l] if kind == "g" else bs[l]
            gb[:, i * T:(i + 1) * T] = np.asarray(src, np.float32)
        base["gb"] = gb

    in_maps = []
    for c in range(NCORES):
        m = dict(base)
        m["x"] = np.ascontiguousarray(x[c * BPC:(c + 1) * BPC])
        if not flags.mask_ones:
            msk = mask[c * BPC:(c + 1) * BPC]             # [BPC, T]
            mb = np.where(msk == 0.0, np.float32(-1e9), np.float32(0.0))
            # [128, BPC*8]: column (b*8+j) = bias for key chunk j of batch b
            m["mb"] = np.ascontiguousarray(
                mb.reshape(BPC, 8, C).transpose(2, 0, 1).reshape(C, BPC * 8))
            m["qm"] = np.ascontiguousarray(msk)
        in_maps.append(m)

    LAST_RUN["nc"] = nc
    LAST_RUN["in_maps"] = in_maps

    res = run_bass_kernel_spmd(nc, in_maps, list(range(NCORES)))
    out = np.concatenate([r["out"] for r in res.results], axis=0)
    return out.astype(np.float32)



# revision 3
# speedup vs baseline: 1.1515x; 1.1515x over previous
"""Trainium2 Bass kernel for nn_Encoder_Block (dense transformer encoder block).

Strategy: pure data parallel across 8 NeuronCores (B=16 -> 2 batch elems per
core), all weights replicated.  Entire block computed on-chip per batch elem:
  x + pos -> LN0 (res) -> 4x [dsconv -> relu -> +res -> LN] -> attention
  -> +res -> LNe -> FC -> relu -> +res

Host-side weight folding (constant prep, input-independent layout changes):
  - depthwise(k=7) + pointwise conv folded into 7 full [C,C] matmul taps:
      W2[c,o; tap d] = pw_w[o,c] * dw_w[c,d]    (accumulated in PSUM over d)
  - conv bias folded: b2 = pw_w @ dw_b + pw_b
  - attention score matrix folded: M = Wq @ Wk^T / sqrt(dk), so the scores
    are computed as s^T = x^T (M^T x) with no separate q/k heads on chip
  - both (identical) heads folded: Wo_eff = Wo[:dk] + Wo[dk:]
  - positional-encoding table precomputed (input-independent constant)

Data plane is bf16 (weights, activations, residuals); statistics, PSUM
accumulation and the final output stay fp32.  Matmuls run bf16 (1 cycle/row
at any moving size).  LayerNorm over (C,T) jointly: per-partition sums via
accum_out on the producing ops, cross-partition reduction via a single
gpsimd partition_all_reduce on the otherwise-idle Pool engine; rsqrt via
exp(-0.5*ln(var+eps)) so the whole kernel stays in one scalar-engine table
set (natural_log_exp_and_others).  Softmax: scores computed transposed
[tk,tq]; exp without max-subtraction (values bounded, safe); denominator via
an appended ones-column in the PV matmul; reciprocal on DVE, broadcast to
partitions via gpsimd partition_broadcast (Pool).
"""

import sys

sys.path.insert(0, "/opt/trn_rl_repo")

import math

import numpy as np
import ml_dtypes

import concourse.bass as bass
import concourse.bass_isa as bass_isa
import concourse.tile as tile
from concourse import bacc, library_config, mybir
from concourse.bass_utils import run_bass_kernel_spmd

F32 = mybir.dt.float32
BF16 = mybir.dt.bfloat16
AF = mybir.ActivationFunctionType
ALU = mybir.AluOpType

B, C, T = 16, 128, 1024
NCONV, KW = 4, 7
DK = C // 2
NCORES = 8
BPC = B // NCORES          # batch elems per core
EPS = 1e-5
NEL = float(C * T)         # layernorm element count
PADT = T + KW - 1          # 1030: per-batch padded row in xpad
NLN = NCONV + 2            # LN0, 4 conv LNs, LNe
H = 512                    # half-width of T


def _pos_encoding() -> np.ndarray:
    i = np.arange(C)
    exp = -((i - (i % 2)).astype(np.float32) / np.float32(C))
    freqs = (np.float32(10000.0) ** exp)[:, None].astype(np.float32)
    phases = ((i % 2).astype(np.float32) * np.float32(np.pi / 2))[:, None]
    pos = np.arange(T, dtype=np.float32)[None, :]
    return np.sin(pos * freqs + phases).astype(np.float32)


def _uniform_val(a: np.ndarray):
    """Return the scalar if all elements equal, else None."""
    v = a.flat[0]
    return float(v) if np.all(a == v) else None


def _bf(a: np.ndarray) -> np.ndarray:
    return np.ascontiguousarray(a.astype(ml_dtypes.bfloat16))


class _Flags:
    """Kernel-structure flags derived from host inspection of the inputs."""

    def __init__(self, ln_gc, ln_bc, b2_zero, fcb_zero, mask_ones):
        self.ln_gc = tuple(ln_gc)
        self.ln_bc = tuple(ln_bc)
        self.b2_zero = b2_zero
        self.fcb_zero = fcb_zero
        self.mask_ones = mask_ones

    def key(self):
        return (self.ln_gc, self.ln_bc, self.b2_zero, self.fcb_zero,
                self.mask_ones)


class _Bacc(bacc.Bacc):
    """Bacc with activation-table choice pinned to the one set that covers
    every function this kernel uses (ln/exp/square/copy/relu/identity).

    The stock insertion pass greedily picks the first act_info set containing
    each function, which lands exp/square in `exp_and_others` and ln in
    `natural_log` — ping-ponging ~2 table loads (~5us) per layernorm.  Hiding
    our functions from every other set (ids/positions unchanged) forces all
    loads to `natural_log_exp_and_others`, so exactly one load is emitted.
    """

    _OURS = {AF.Ln, AF.Exp, AF.Square, AF.Copy, AF.Identity, AF.Relu}
    _KEEP = "natural_log_exp_and_others"

    def insert_act_table_loads(self):
        from concourse.bacc import _bass_rust, get_activation_tables
        has_activation = any(
            isinstance(i, mybir.InstActivation)
            for b in self.main_func.blocks
            for i in b.instructions
        )
        if not has_activation:
            return
        tables = [
            (nm, fs if nm == self._KEEP else (fs - self._OURS))
            for nm, fs in get_activation_tables(self.m.arch).items()
        ]
        _bass_rust.insert_act_table_loads(self, tables)


def _build(flags: _Flags):
    nc = _Bacc("TRN2", target_bir_lowering=False, debug=False,
               num_devices=NCORES)

    def dram(name, shape, dtype=F32, kind="ExternalInput"):
        return nc.dram_tensor(name, shape, dtype, kind=kind).ap()

    x_d = dram("x", [BPC, C, T])
    pos_d = dram("pos", [C, T])
    w2_d = dram("w2", [C, NCONV * KW * C], BF16)
    m_d = dram("mqk", [C, C], BF16)
    wv_d = dram("wv", [C, DK], BF16)
    wo_d = dram("wo", [DK, C], BF16)
    fcw_d = dram("fcw", [C, C], BF16)
    out_d = dram("out", [BPC, C, T], kind="ExternalOutput")
    if not flags.b2_zero:
        b2_d = dram("b2", [C, NCONV])
    if not flags.fcb_zero:
        fcb_d = dram("fcb", [C, 1])
    nonuni = [l for l in range(NLN)
              if flags.ln_gc[l] is None or flags.ln_bc[l] is None]
    gb_entries = ([(l, "g") for l in range(NLN) if flags.ln_gc[l] is None]
                  + [(l, "b") for l in range(NLN) if flags.ln_bc[l] is None])
    if gb_entries:
        gb_d = dram("gb", [C, len(gb_entries) * T], BF16)
    if not flags.mask_ones:
        mb_d = dram("mb", [C, BPC * (T // C)])   # [128, 2*8] key-mask exp bias
        qm_d = dram("qm", [BPC, T])              # query-mask rows

    from contextlib import ExitStack

    with tile.TileContext(nc) as tc, ExitStack() as ctx:
        cst = ctx.enter_context(tc.tile_pool(name="cst", bufs=1))
        big = ctx.enter_context(tc.tile_pool(name="bigbuf", bufs=1))
        resp = ctx.enter_context(tc.tile_pool(name="resp", bufs=2))
        work = ctx.enter_context(tc.tile_pool(name="work", bufs=2))
        tiny = ctx.enter_context(tc.tile_pool(name="tiny", bufs=4))
        ps_big = ctx.enter_context(
            tc.tile_pool(name="ps_big", bufs=2, space="PSUM"))
        ps_sm = ctx.enter_context(
            tc.tile_pool(name="ps_sm", bufs=2, space="PSUM"))
        ps_av = ctx.enter_context(
            tc.tile_pool(name="ps_av", bufs=1, space="PSUM"))

        nc.gpsimd.load_library(library_config.attn)

        # ---- constants / weights in SBUF ----
        const_tiles: dict = {}

        def const_ap(val: float, npart: int = C):
            """[npart,1] fp32 SBUF constant (activation bias operand)."""
            if val == 0.0:
                return 0.0   # pre-registered const AP
            if val not in const_tiles:
                t = cst.tile([C, 1], F32, tag=f"cst{len(const_tiles)}")
                nc.vector.memset(t[:], val)
                const_tiles[val] = t
            return const_tiles[val][0:npart, :]

        # prologue DMAs spread across per-engine HWDGE queues so the
        # x / pos / weight loads stream in parallel
        xin = big.tile([C, BPC * T], F32, tag="xin")
        nc.sync.dma_start(xin[:, 0:T], x_d[0])
        nc.scalar.dma_start(xin[:, T:2 * T], x_d[1])
        pos_sb = cst.tile([C, T], F32, tag="pos")
        nc.gpsimd.dma_start(pos_sb[:], pos_d[:])
        w2_sb = cst.tile([C, NCONV * KW * C], BF16, tag="w2")
        LW = KW * C
        nc.sync.dma_start(w2_sb[:, 0:LW], w2_d[:, 0:LW])
        nc.scalar.dma_start(w2_sb[:, LW:2 * LW], w2_d[:, LW:2 * LW])
        nc.gpsimd.dma_start(w2_sb[:, 2 * LW:3 * LW], w2_d[:, 2 * LW:3 * LW])
        nc.sync.dma_start(w2_sb[:, 3 * LW:4 * LW], w2_d[:, 3 * LW:4 * LW])
        m_sb = cst.tile([C, C], BF16, tag="mqk")
        nc.gpsimd.dma_start(m_sb[:], m_d)
        wv_sb = cst.tile([C, DK], BF16, tag="wv")
        nc.gpsimd.dma_start(wv_sb[:], wv_d)
        wo_sb = cst.tile([DK, C], BF16, tag="wo")
        nc.scalar.dma_start(wo_sb[:], wo_d)
        fcw_sb = cst.tile([C, C], BF16, tag="fcw")
        nc.scalar.dma_start(fcw_sb[:], fcw_d)
        if not flags.b2_zero:
            b2_sb = cst.tile([C, NCONV], F32, tag="b2")
            nc.sync.dma_start(b2_sb[:], b2_d[:])
        if not flags.fcb_zero:
            fcb_sb = cst.tile([C, 1], F32, tag="fcb")
            nc.sync.dma_start(fcb_sb[:], fcb_d[:])
        if gb_entries:
            gb_sb = cst.tile([C, len(gb_entries) * T], BF16, tag="gb")
            nc.sync.dma_start(gb_sb[:], gb_d[:])
            gb_ix = {e: i for i, e in enumerate(gb_entries)}

            def gb_ap(l, kind):
                i0 = gb_ix[(l, kind)] * T
                return gb_sb[:, i0:i0 + T]
        if not flags.mask_ones:
            mb_sb = cst.tile([C, BPC * (T // C)], F32, tag="mb")
            nc.sync.dma_start(mb_sb[:], mb_d[:])
            qm_sb = cst.tile([1, BPC * T], F32, tag="qm")
            for b in range(BPC):
                nc.sync.dma_start(qm_sb[:, b * T:(b + 1) * T], qm_d[b:b + 1, :])

        # conv/attention input activations, zero-padded per batch elem:
        # [0^3 | x_b (1024) | 0^3] so every conv tap is a full N=512 matmul.
        xpad = big.tile([C, BPC * PADT], BF16, tag="xpad")
        for b in range(BPC):
            nc.vector.memset(xpad[:, b * PADT:b * PADT + 3], 0.0)
            nc.vector.memset(xpad[:, b * PADT + 3 + T:(b + 1) * PADT], 0.0)

        out_sb = big.tile([C, BPC * T], F32, tag="out_sb")

        def ip(b, off=0, n=T):
            """AP of the xpad interior for batch b (bf16)."""
            return xpad[:, b * PADT + 3 + off: b * PADT + 3 + off + n]

        def emit_ln_tail(ln_idx, b, stats, src, dst):
            """Finish LayerNorm over (C,T) for one batch elem.

            stats: [C,4] fp32 tile holding per-partition partial sums
            (S1_h0, S2_h0, S1_h1, S2_h1) from the producing ops' accum_out.
            src: bf16 [C,T] AP (the residual-stream value to normalize);
            dst: bf16 [C,T] AP (LN output, feeds matmuls).
            """
            red = tiny.tile([C, 4], F32, tag="red")
            nc.gpsimd.partition_all_reduce(red[:], stats[:], channels=C,
                                           reduce_op=bass_isa.ReduceOp.add)
            # s12 = [S1, S2] (halves combined); mom = [mu, m2]
            mom = tiny.tile([C, 2], F32, tag="mom")
            nc.vector.tensor_tensor(mom[:], red[:, 0:2], red[:, 2:4], ALU.add)
            nc.vector.tensor_scalar_mul(mom[:], mom[:], 1.0 / NEL)
            musq = tiny.tile([C, 1], F32, tag="musq")
            nc.vector.tensor_tensor(musq[:], mom[:, 0:1], mom[:, 0:1],
                                    ALU.mult)
            # vare = (m2 + eps) - mu^2
            vare = tiny.tile([C, 1], F32, tag="vare")
            nc.vector.scalar_tensor_tensor(vare[:], mom[:, 1:2], EPS, musq[:],
                                           op0=ALU.add, op1=ALU.subtract)
            # rs = gc / sqrt(var+eps) = exp(-0.5*ln(var+eps) + ln(gc));
            # stays in the natural_log_exp_and_others ACT table set.
            gc, bc = flags.ln_gc[ln_idx], flags.ln_bc[ln_idx]
            lnv = tiny.tile([C, 1], F32, tag="lnv")
            nc.scalar.activation(lnv[:], vare[:], AF.Ln)
            rs = tiny.tile([C, 1], F32, tag="rs")
            expb = math.log(gc) if (gc is not None and gc > 0.0) else 0.0
            nc.scalar.activation(rs[:], lnv[:], AF.Exp, scale=-0.5,
                                 bias=const_ap(expb))
            if gc is not None and gc <= 0.0 and gc != 1.0:
                rs2 = tiny.tile([C, 1], F32, tag="rs2")
                nc.vector.tensor_scalar_mul(rs2[:], rs[:], gc)
                rs = rs2
            mu = mom[:, 0:1]
            post = []
            if gc is None:
                post.append(lambda i, o: nc.vector.tensor_tensor(
                    o, i, gb_ap(ln_idx, "g"), ALU.mult))
            if bc is None:
                post.append(lambda i, o: nc.vector.tensor_tensor(
                    o, i, gb_ap(ln_idx, "b"), ALU.add))
            elif bc != 0.0:
                post.append(lambda i, o: nc.vector.tensor_scalar_add(
                    o, i, bc))
            if not post:
                # per half so the consumer's first-half matmuls start early
                for h in range(2):
                    sl = slice(h * H, (h + 1) * H)
                    nc.vector.tensor_scalar(dst[:, sl], src[:, sl], mu,
                                            rs[:], op0=ALU.subtract,
                                            op1=ALU.mult)
            else:
                mids = [work.tile([C, T], BF16, tag="lnmid", bufs=2),
                        work.tile([C, T], BF16, tag="lnmid2", bufs=2)]
                nc.vector.tensor_scalar(mids[0][:], src, mu, rs[:],
                                        op0=ALU.subtract, op1=ALU.mult)
                for i, emit in enumerate(post):
                    last = (i == len(post) - 1)
                    emit(mids[i % 2][:], dst if last else mids[(i + 1) % 2][:])

        # ---- x + pos -> xpad (conv0 input); LN0 -> res ----
        res = resp.tile([C, BPC * T], BF16, tag="res")
        for b in range(BPC):
            stats = tiny.tile([C, 4], F32, tag="stats")
            scr = work.tile([C, T], BF16, tag="sqscr")
            for h in range(2):
                sl = slice(h * H, (h + 1) * H)
                nc.vector.scalar_tensor_tensor(
                    ip(b, h * H, H), xin[:, b * T + h * H:b * T + (h + 1) * H],
                    1.0, pos_sb[:, sl],
                    op0=ALU.mult, op1=ALU.add, accum_out=stats[:, 2 * h:2 * h + 1])
                nc.scalar.activation(scr[:, sl], ip(b, h * H, H), AF.Square,
                                     accum_out=stats[:, 2 * h + 1:2 * h + 2])
            emit_ln_tail(0, b, stats, ip(b), res[:, b * T:(b + 1) * T])

        # ---- conv layers ----
        for li in range(NCONV):
            new_res = resp.tile([C, BPC * T], BF16, tag="res")
            for b in range(BPC):
                stats = tiny.tile([C, 4], F32, tag="stats")
                scr = work.tile([C, T], BF16, tag="sqscr")
                psc = ps_big.tile([C, T], F32, tag="ps_main")
                nr = new_res[:, b * T:(b + 1) * T]
                for h in range(2):
                    for d in range(KW):
                        nc.tensor.matmul(
                            psc[:, h * H:(h + 1) * H],
                            w2_sb[:, (li * KW + d) * C:(li * KW + d + 1) * C],
                            xpad[:, b * PADT + h * H + d:
                                 b * PADT + h * H + d + H],
                            start=(d == 0), stop=(d == KW - 1))
                    sl = slice(h * H, (h + 1) * H)
                    if flags.b2_zero:
                        nc.vector.scalar_tensor_tensor(
                            nr[:, sl], psc[:, sl], 0.0, res[:, b * T + h * H:
                                                           b * T + (h + 1) * H],
                            op0=ALU.max, op1=ALU.add,
                            accum_out=stats[:, 2 * h:2 * h + 1])
                    else:
                        relu_t = work.tile([C, T], BF16, tag="relu_t")
                        nc.scalar.activation(relu_t[:, sl], psc[:, sl],
                                             AF.Relu, bias=b2_sb[:, li:li + 1])
                        nc.vector.scalar_tensor_tensor(
                            nr[:, sl], relu_t[:, sl], 1.0,
                            res[:, b * T + h * H:b * T + (h + 1) * H],
                            op0=ALU.mult, op1=ALU.add,
                            accum_out=stats[:, 2 * h:2 * h + 1])
                    nc.scalar.activation(scr[:, sl], nr[:, sl], AF.Square,
                                         accum_out=stats[:, 2 * h + 1:2 * h + 2])
                emit_ln_tail(1 + li, b, stats, nr, ip(b))
            res = new_res

        # ---- attention (input: normalized, in xpad interior) ----
        new_res = resp.tile([C, BPC * T], BF16, tag="res")
        for b in range(BPC):
            stats = tiny.tile([C, 4], F32, tag="stats")
            xa = ip(b)  # [128, 1024] bf16
            # y = M^T x  (scores s^T[tk,tq] = sum_c' x[c',tk] y[c',tq])
            psy = ps_big.tile([C, T], F32, tag="ps_main")
            y_sb = work.tile([C, T], BF16, tag="y_sb")
            for h in range(2):
                sl = slice(h * H, (h + 1) * H)
                nc.tensor.matmul(psy[:, sl], m_sb[:], xa[:, sl],
                                 start=True, stop=True)
            nc.vector.tensor_copy(y_sb[:, 0:H], psy[:, 0:H])
            nc.scalar.copy(y_sb[:, H:T], psy[:, H:T])
            # v in [t, d] layout + appended ones column (denominator)
            psv = ps_sm.tile([C, H], F32, tag="ps_sm")
            for j in range(8):
                nc.tensor.matmul(psv[:, j * DK:(j + 1) * DK],
                                 xa[:, j * C:(j + 1) * C], wv_sb[:],
                                 start=True, stop=True)
            vt = work.tile([C, 8, DK + 1], BF16, tag="vt")
            nc.vector.memset(vt[:, :, DK:DK + 1], 1.0)
            nc.vector.tensor_copy(
                vt[:, :, 0:DK],
                psv[:].rearrange("p (j k) -> p j k", k=DK))
            # scores (transposed: [tk, tq]) + exp, per key chunk
            eT = work.tile([C, 8 * T], BF16, tag="eT", bufs=2)
            for j in range(8):
                pss = ps_big.tile([C, T], F32, tag="ps_main")
                for h in range(2):
                    nc.tensor.matmul(pss[:, h * H:(h + 1) * H],
                                     xa[:, j * C:(j + 1) * C],
                                     y_sb[:, h * H:(h + 1) * H],
                                     start=True, stop=True)
                if flags.mask_ones:
                    nc.scalar.activation(eT[:, j * T:(j + 1) * T], pss[:],
                                         AF.Exp)
                else:
                    nc.scalar.activation(eT[:, j * T:(j + 1) * T], pss[:],
                                         AF.Exp,
                                         bias=mb_sb[:, b * 8 + j:b * 8 + j + 1])
            # attention-weighted values + denominator (appended ones column)
            psa = ps_av.tile([DK + 1, T], F32, tag="ps_av")
            for h in range(2):
                for j in range(8):
                    nc.tensor.matmul(
                        psa[:, h * H:(h + 1) * H], vt[:, j, :],
                        eT[:, j * T + h * H: j * T + h * H + H],
                        start=(j == 0), stop=(j == 7))
            # Per-half tail: reciprocal of the denominator row on DVE,
            # partition-broadcast on Pool, normalize+cast on DVE, Wo matmul.
            rr = tiny.tile([1, T], F32, tag="rr", bufs=2)
            rrb = work.tile([DK, T], F32, tag="rrb", bufs=2)
            avn = work.tile([DK, T], BF16, tag="avn")
            nr = new_res[:, b * T:(b + 1) * T]
            scr = work.tile([C, T], BF16, tag="sqscr")
            for h in range(2):
                sl = slice(h * H, (h + 1) * H)
                if flags.mask_ones:
                    nc.vector.reciprocal(rr[:, sl], psa[DK:DK + 1, sl])
                else:
                    # guard fully-masked queries (denom==0 -> finite rr; av
                    # rows are 0 so the product stays 0)
                    nc.vector.tensor_scalar_add(rr[:, sl],
                                                psa[DK:DK + 1, sl], 1e-30)
                    nc.vector.reciprocal(rr[:, sl], rr[:, sl])
                nc.gpsimd.partition_broadcast(rrb[:, sl], rr[:, sl],
                                              channels=DK)
                nc.vector.tensor_tensor(avn[:, sl], psa[0:DK, sl],
                                        rrb[:, sl], ALU.mult)
                pso = ps_sm.tile([C, H], F32, tag="ps_sm")
                nc.tensor.matmul(pso[:], wo_sb[:], avn[:, sl],
                                 start=True, stop=True)
                rsl = slice(b * T + h * H, b * T + (h + 1) * H)
                if flags.mask_ones:
                    nc.vector.scalar_tensor_tensor(
                        nr[:, sl], pso[:], 1.0, res[:, rsl],
                        op0=ALU.mult, op1=ALU.add,
                        accum_out=stats[:, 2 * h:2 * h + 1])
                else:
                    # query-mask rows broadcast across partitions (Pool),
                    # applied before the residual add
                    qmb = work.tile([C, T], F32, tag="qmb")
                    att = work.tile([C, T], BF16, tag="att_m")
                    nc.gpsimd.partition_broadcast(
                        qmb[:, sl], qm_sb[:, b * T + h * H:b * T + (h + 1) * H],
                        channels=C)
                    nc.vector.tensor_tensor(att[:, sl], pso[:], qmb[:, sl],
                                            ALU.mult)
                    nc.vector.scalar_tensor_tensor(
                        nr[:, sl], att[:, sl], 1.0, res[:, rsl],
                        op0=ALU.mult, op1=ALU.add,
                        accum_out=stats[:, 2 * h:2 * h + 1])
                # square on DVE here: ACT is saturated by softmax exp
                nc.vector.scalar_tensor_tensor(
                    scr[:, sl], nr[:, sl], 1.0, nr[:, sl],
                    op0=ALU.mult, op1=ALU.mult,
                    accum_out=stats[:, 2 * h + 1:2 * h + 2])
            emit_ln_tail(NLN - 1, b, stats, nr, ip(b))
        res = new_res

        # ---- final FC + relu + residual ----
        for b in range(BPC):
            psf = ps_big.tile([C, T], F32, tag="ps_main")
            for h in range(2):
                nc.tensor.matmul(psf[:, h * H:(h + 1) * H], fcw_sb[:],
                                 ip(b, h * H, H), start=True, stop=True)
            for h in range(2):
                sl = slice(h * H, (h + 1) * H)
                ob = out_sb[:, b * T + h * H:b * T + (h + 1) * H]
                rb_ = res[:, b * T + h * H:b * T + (h + 1) * H]
                if flags.fcb_zero:
                    nc.vector.scalar_tensor_tensor(
                        ob, psf[:, sl], 0.0, rb_, op0=ALU.max, op1=ALU.add)
                else:
                    relu_t = work.tile([C, T], BF16, tag="relu_t")
                    nc.scalar.activation(relu_t[:, sl], psf[:, sl], AF.Relu,
                                         bias=fcb_sb[:])
                    nc.vector.tensor_tensor(ob, relu_t[:, sl], rb_, ALU.add)
                (nc.sync if h == 0 else nc.scalar).dma_start(
                    out_d[b][:, sl], ob)

    nc.compile()
    return nc


_CACHE: dict = {}
LAST_RUN: dict = {}   # exposed for test harnesses (nc, in_maps)


def kernel(x, mask, dw_w, dw_b, pw_w, pw_b, norm0_g, norm0_b,
           norms_g, norms_b, norme_g, norme_b,
           Wq, Wk, Wv, Wo, fc_w, fc_b):
    x = np.asarray(x, dtype=np.float32)
    mask = np.asarray(mask, dtype=np.float32)

    # ---- host-side constant folding ----
    w2 = np.empty((C, NCONV, KW, C), dtype=np.float32)
    for i in range(NCONV):
        pwT = np.asarray(pw_w[i], np.float32).T          # [c, o]
        for d in range(KW):
            w2[:, i, d, :] = pwT * np.asarray(dw_w[i][:, d],
                                              np.float32)[:, None]
    w2 = w2.reshape(C, NCONV * KW * C)
    b2 = np.stack([np.asarray(pw_w[i], np.float32)
                   @ np.asarray(dw_b[i], np.float32)
                   + np.asarray(pw_b[i], np.float32)
                   for i in range(NCONV)], axis=1)        # [C, NCONV]
    mqk = (np.asarray(Wq, np.float32) @ np.asarray(Wk, np.float32).T
           / math.sqrt(DK))                               # [C, C]
    wo = np.asarray(Wo, np.float32)
    wo_eff = np.ascontiguousarray(wo[:DK] + wo[DK:])      # [DK, C]
    fcw = np.ascontiguousarray(np.asarray(fc_w, np.float32).T)  # [c, o]
    fcb = np.asarray(fc_b, np.float32).reshape(C, 1)
    pos = _pos_encoding()

    gs = [norm0_g] + [norms_g[i] for i in range(NCONV)] + [norme_g]
    bs = [norm0_b] + [norms_b[i] for i in range(NCONV)] + [norme_b]
    ln_gc = [_uniform_val(np.asarray(g, np.float32)) for g in gs]
    ln_bc = [_uniform_val(np.asarray(bb, np.float32)) for bb in bs]
    flags = _Flags(ln_gc, ln_bc,
                   b2_zero=not b2.any(),
                   fcb_zero=not fcb.any(),
                   mask_ones=bool(np.all(mask == 1.0)))

    key = flags.key()
    if key not in _CACHE:
        _CACHE[key] = _build(flags)
    nc = _CACHE[key]

    base = {"pos": pos, "w2": _bf(w2), "mqk": _bf(mqk),
            "wv": _bf(np.asarray(Wv, np.float32)),
            "wo": _bf(wo_eff), "fcw": _bf(fcw)}
    if not flags.b2_zero:
        base["b2"] = np.ascontiguousarray(b2)
    if not flags.fcb_zero:
        base["fcb"] = np.ascontiguousarray(fcb)
    gb_entries = ([(l, "g") for l in range(NLN) if flags.ln_gc[l] is None]
                  + [(l, "b") for l in range(NLN) if flags.ln_bc[l] is None])
    if gb_entries:
        gb = np.empty((C, len(gb_entries) * T), np.float32)
        for i, (l, kind) in enumerate(gb_entries):
            src = gs[l] if kind == "g" else bs[l]
            gb[:, i * T:(i + 1) * T] = np.asarray(src, np.float32)
        base["gb"] = _bf(gb)

    in_maps = []
    for c in range(NCORES):
        m = dict(base)
        m["x"] = np.ascontiguousarray(x[c * BPC:(c + 1) * BPC])
        if not flags.mask_ones:
            msk = mask[c * BPC:(c + 1) * BPC]             # [BPC, T]
            mb = np.where(msk == 0.0, np.float32(-1e9), np.float32(0.0))
            # [128, BPC*8]: column (b*8+j) = bias for key chunk j of batch b
            m["mb"] = np.ascontiguousarray(
                mb.reshape(BPC, 8, C).transpose(2, 0, 1).reshape(C, BPC * 8))
            m["qm"] = np.ascontiguousarray(msk)
        in_maps.append(m)

    LAST_RUN["nc"] = nc
    LAST_RUN["in_maps"] = in_maps

    res = run_bass_kernel_spmd(nc, in_maps, list(range(NCORES)))
    out = np.concatenate([r["out"] for r in res.results], axis=0)
    return out.astype(np.float32)


# revision 18
# speedup vs baseline: 1.2619x; 1.0958x over previous
"""Trainium2 Bass kernel for nn_Encoder_Block (dense transformer encoder block).

Strategy: pure data parallel across 8 NeuronCores (B=16 -> 2 batch elems per
core), all weights replicated.  Entire block computed on-chip per batch elem:
  x + pos -> LN0 (res) -> 4x [dsconv -> relu -> +res -> LN] -> attention
  -> +res -> LNe -> FC -> relu -> +res

Host-side weight folding (constant prep, input-independent layout changes):
  - depthwise(k=7) + pointwise conv folded into 7 full [C,C] matmul taps:
      W2[c,o; tap d] = pw_w[o,c] * dw_w[c,d]    (accumulated in PSUM over d)
  - conv bias folded: b2 = pw_w @ dw_b + pw_b
  - attention score matrix folded: M = Wq @ Wk^T / sqrt(dk), so the scores
    are computed as s^T = x^T (M^T x) with no separate q/k heads on chip
  - both (identical) heads folded: Wo_eff = Wo[:dk] + Wo[dk:]
  - positional-encoding table precomputed (input-independent constant)

Data plane is bf16 (weights, activations, residuals); statistics, PSUM
accumulation and the final output stay fp32.  LayerNorm over (C,T) jointly:
per-partition sums via accum_out on the producing ops, cross-partition
reduction via a single gpsimd partition_all_reduce on the otherwise-idle
Pool engine; rsqrt via exp(-0.5*ln(var+eps)) so the whole kernel stays in
one scalar-engine table set (natural_log_exp_and_others).  Softmax: scores
computed transposed [tk,tq]; exp without max-subtraction (values bounded,
safe); denominator via an appended ones-column in the PV matmul; reciprocal
on DVE, broadcast to partitions via gpsimd partition_broadcast (Pool).

LayerNorm outputs are written in chunks [0:520] / [520:1024] so the next
conv layer's first-half taps (which read padded cols 0..518) start as soon
as the first chunk lands.  PSUM rings: conv accumulators rotate through a
2-deep [C,512] ring; each batch's attention/FC PSUM rotates through its own
2-deep [C,512] ring so the two batches' attention phases overlap.
"""

import sys

sys.path.insert(0, "/opt/trn_rl_repo")

import math

import numpy as np
import ml_dtypes

import concourse.bass as bass
import concourse.bass_isa as bass_isa
import concourse.tile as tile
from concourse import bacc, library_config, mybir
from concourse.bass_utils import run_bass_kernel_spmd

F32 = mybir.dt.float32
BF16 = mybir.dt.bfloat16
AF = mybir.ActivationFunctionType
ALU = mybir.AluOpType

B, C, T = 16, 128, 1024
NCONV, KW = 4, 7
DK = C // 2
NCORES = 8
BPC = B // NCORES          # batch elems per core
EPS = 1e-5
NEL = float(C * T)         # layernorm element count
PADT = T + KW - 1          # 1030: per-batch padded row in xpad
NLN = NCONV + 2            # LN0, 4 conv LNs, LNe
H = 512                    # half-width of T
NCH = 520                  # norm chunk boundary (covers conv h0 tap reads)


def _pos_encoding() -> np.ndarray:
    i = np.arange(C)
    exp = -((i - (i % 2)).astype(np.float32) / np.float32(C))
    freqs = (np.float32(10000.0) ** exp)[:, None].astype(np.float32)
    phases = ((i % 2).astype(np.float32) * np.float32(np.pi / 2))[:, None]
    pos = np.arange(T, dtype=np.float32)[None, :]
    return np.sin(pos * freqs + phases).astype(np.float32)


def _uniform_val(a: np.ndarray):
    """Return the scalar if all elements equal, else None."""
    v = a.flat[0]
    return float(v) if np.all(a == v) else None


def _bf(a: np.ndarray) -> np.ndarray:
    return np.ascontiguousarray(a.astype(ml_dtypes.bfloat16))


class _Flags:
    """Kernel-structure flags derived from host inspection of the inputs."""

    def __init__(self, ln_gc, ln_bc, b2_zero, fcb_zero, mask_ones):
        self.ln_gc = tuple(ln_gc)
        self.ln_bc = tuple(ln_bc)
        self.b2_zero = b2_zero
        self.fcb_zero = fcb_zero
        self.mask_ones = mask_ones

    def key(self):
        return (self.ln_gc, self.ln_bc, self.b2_zero, self.fcb_zero,
                self.mask_ones)


class _Bacc(bacc.Bacc):
    """Bacc with activation-table choice pinned to the one set that covers
    every function this kernel uses (ln/exp/square/copy/relu/identity).

    The stock insertion pass greedily picks the first act_info set containing
    each function, which lands exp/square in `exp_and_others` and ln in
    `natural_log` — ping-ponging ~2 table loads (~5us) per layernorm.  Hiding
    our functions from every other set (ids/positions unchanged) forces all
    loads to `natural_log_exp_and_others`, so exactly one load is emitted.
    """

    _OURS = {AF.Ln, AF.Exp, AF.Square, AF.Copy, AF.Identity, AF.Relu}
    _KEEP = "natural_log_exp_and_others"

    def insert_act_table_loads(self):
        from concourse.bacc import _bass_rust, get_activation_tables
        has_activation = any(
            isinstance(i, mybir.InstActivation)
            for b in self.main_func.blocks
            for i in b.instructions
        )
        if not has_activation:
            return
        tables = [
            (nm, fs if nm == self._KEEP else (fs - self._OURS))
            for nm, fs in get_activation_tables(self.m.arch).items()
        ]
        _bass_rust.insert_act_table_loads(self, tables)


def _build(flags: _Flags):
    nc = _Bacc("TRN2", target_bir_lowering=False, debug=False,
               num_devices=NCORES)

    def dram(name, shape, dtype=F32, kind="ExternalInput"):
        return nc.dram_tensor(name, shape, dtype, kind=kind).ap()

    x_d = dram("x", [C, BPC * T], BF16)       # host pre-transposed [C, b*T]
    pos_d = dram("pos", [C, T], BF16)
    w2_d = dram("w2", [C, NCONV * KW * C], BF16)
    # packed small weights: [mqk | wv | fcw | wo(rows 0:DK)]
    wc_d = dram("wcat", [C, 3 * C + DK], BF16)
    out_d = dram("out", [BPC, C, T], BF16, kind="ExternalOutput")
    if not flags.b2_zero:
        b2_d = dram("b2", [C, NCONV])
    if not flags.fcb_zero:
        fcb_d = dram("fcb", [C, 1])
    gb_entries = ([(l, "g") for l in range(NLN) if flags.ln_gc[l] is None]
                  + [(l, "b") for l in range(NLN) if flags.ln_bc[l] is None])
    if gb_entries:
        gb_d = dram("gb", [C, len(gb_entries) * T], BF16)
    if not flags.mask_ones:
        mb_d = dram("mb", [C, BPC * (T // C)])   # [128, 2*8] key-mask exp bias
        qm_d = dram("qm", [BPC, T])              # query-mask rows

    from contextlib import ExitStack

    with tile.TileContext(nc) as tc, ExitStack() as ctx:
        cst = ctx.enter_context(tc.tile_pool(name="cst", bufs=1))
        big = ctx.enter_context(tc.tile_pool(name="bigbuf", bufs=1))
        resp = ctx.enter_context(tc.tile_pool(name="resp", bufs=2))
        work = ctx.enter_context(tc.tile_pool(name="work", bufs=2))
        tiny = ctx.enter_context(tc.tile_pool(name="tiny", bufs=4))
        ps_cv = ctx.enter_context(
            tc.tile_pool(name="ps_cv", bufs=2, space="PSUM"))
        ps_at = ctx.enter_context(
            tc.tile_pool(name="ps_at", bufs=2, space="PSUM"))
        ps_av = ctx.enter_context(
            tc.tile_pool(name="ps_av", bufs=1, space="PSUM"))

        nc.gpsimd.load_library(library_config.attn)

        # ---- constants / weights in SBUF ----
        const_tiles: dict = {}

        def const_ap(val: float, npart: int = C):
            """[npart,1] fp32 SBUF constant (activation bias operand)."""
            if val == 0.0:
                return 0.0   # pre-registered const AP
            if val not in const_tiles:
                t = cst.tile([C, 1], F32, tag=f"cst{len(const_tiles)}")
                nc.vector.memset(t[:], val)
                const_tiles[val] = t
            return const_tiles[val][0:npart, :]

        # prologue DMAs: x / pos chunked and spread across the three DMA
        # queues (SP + Act HWDGE, gpsimd SWDGE) so LN0 and the first conv's
        # weights arrive as early as possible
        xin = big.tile([C, BPC * T], BF16, tag="xin")
        pos_sb = cst.tile([C, T], BF16, tag="pos")
        w2_sb = cst.tile([C, NCONV * KW * C], BF16, tag="w2")
        wc_sb = cst.tile([C, 3 * C + DK], BF16, tag="wcat")
        LW = KW * C
        nc.sync.dma_start(xin[:], x_d)
        nc.sync.dma_start(w2_sb[:, 0:LW], w2_d[:, 0:LW])
        nc.sync.dma_start(w2_sb[:, LW:NCONV * LW], w2_d[:, LW:NCONV * LW])
        nc.scalar.dma_start(pos_sb[:], pos_d)
        nc.scalar.dma_start(wc_sb[:], wc_d)
        m_sb = wc_sb[:, 0:C]
        wv_sb = wc_sb[:, C:C + DK]
        fcw_sb = wc_sb[:, C + DK:2 * C + DK]
        wo_sb = wc_sb[0:DK, 2 * C + DK:3 * C + DK]
        if not flags.b2_zero:
            b2_sb = cst.tile([C, NCONV], F32, tag="b2")
            nc.sync.dma_start(b2_sb[:], b2_d[:])
        if not flags.fcb_zero:
            fcb_sb = cst.tile([C, 1], F32, tag="fcb")
            nc.sync.dma_start(fcb_sb[:], fcb_d[:])
        if gb_entries:
            gb_sb = cst.tile([C, len(gb_entries) * T], BF16, tag="gb")
            nc.sync.dma_start(gb_sb[:], gb_d[:])
            gb_ix = {e: i for i, e in enumerate(gb_entries)}

            def gb_ap(l, kind):
                i0 = gb_ix[(l, kind)] * T
                return gb_sb[:, i0:i0 + T]
        if not flags.mask_ones:
            mb_sb = cst.tile([C, BPC * (T // C)], F32, tag="mb")
            nc.sync.dma_start(mb_sb[:], mb_d[:])
            qm_sb = cst.tile([1, BPC * T], F32, tag="qm")
            for b in range(BPC):
                nc.sync.dma_start(qm_sb[:, b * T:(b + 1) * T], qm_d[b:b + 1, :])

        # conv/attention input activations, zero-padded per batch elem:
        # [0^3 | x_b (1024) | 0^3] so every conv tap is a full N=512 matmul.
        xpad = big.tile([C, BPC * PADT], BF16, tag="xpad")
        for b in range(BPC):
            nc.vector.memset(xpad[:, b * PADT:b * PADT + 3], 0.0)
            nc.vector.memset(xpad[:, b * PADT + 3 + T:(b + 1) * PADT], 0.0)

        out_sb = big.tile([C, BPC * T], BF16, tag="out_sb")

        def ip(b, off=0, n=T):
            """AP of the xpad interior for batch b (bf16)."""
            return xpad[:, b * PADT + 3 + off: b * PADT + 3 + off + n]

        def emit_ln_tail(ln_idx, b, stats, src, dst):
            """Finish LayerNorm over (C,T) for one batch elem.

            stats: [C,4] fp32 tile holding per-partition partial sums
            (S1_h0, S2_h0, S1_h1, S2_h1) from the producing ops' accum_out.
            src: bf16 [C,T] AP (the residual-stream value to normalize);
            dst: bf16 [C,T] AP (LN output, feeds matmuls).
            """
            red = tiny.tile([C, 4], F32, tag="red")
            nc.gpsimd.partition_all_reduce(red[:], stats[:], channels=C,
                                           reduce_op=bass_isa.ReduceOp.add)
            # mom = [U, V] = total [sum, sum-of-squares] (halves combined)
            mom = tiny.tile([C, 2], F32, tag="mom")
            nc.vector.tensor_tensor(mom[:], red[:, 0:2], red[:, 2:4], ALU.add)
            # musq = U^2/N ; vare = (V + N*eps) - U^2/N = N*(var+eps)
            musq = tiny.tile([C, 1], F32, tag="musq")
            nc.vector.scalar_tensor_tensor(musq[:], mom[:, 0:1], 1.0 / NEL,
                                           mom[:, 0:1],
                                           op0=ALU.mult, op1=ALU.mult)
            vare = tiny.tile([C, 1], F32, tag="vare")
            nc.vector.scalar_tensor_tensor(vare[:], mom[:, 1:2], NEL * EPS,
                                           musq[:],
                                           op0=ALU.add, op1=ALU.subtract)
            # rs = gc / sqrt(var+eps) = exp(-0.5*ln(N*(var+eps)) + 0.5*ln N
            #      + ln(gc)); stays in the natural_log_exp ACT table set.
            gc, bc = flags.ln_gc[ln_idx], flags.ln_bc[ln_idx]
            lnv = tiny.tile([C, 1], F32, tag="lnv")
            nc.scalar.activation(lnv[:], vare[:], AF.Ln)
            rs = tiny.tile([C, 1], F32, tag="rs")
            expb = (0.5 * math.log(NEL)
                    + (math.log(gc) if (gc is not None and gc > 0.0) else 0.0))
            nc.scalar.activation(rs[:], lnv[:], AF.Exp, scale=-0.5,
                                 bias=const_ap(expb))
            if gc is not None and gc <= 0.0 and gc != 1.0:
                rs2 = tiny.tile([C, 1], F32, tag="rs2")
                nc.vector.tensor_scalar_mul(rs2[:], rs[:], gc)
                rs = rs2
            # murs = (U/N)*rs so normalize is x*rs - murs (both 4x-capable)
            murs = tiny.tile([C, 1], F32, tag="murs")
            nc.vector.scalar_tensor_tensor(murs[:], mom[:, 0:1], 1.0 / NEL,
                                           rs[:],
                                           op0=ALU.mult, op1=ALU.mult)
            post = []
            if gc is None:
                post.append(lambda i, o: nc.vector.tensor_tensor(
                    o, i, gb_ap(ln_idx, "g"), ALU.mult))
            if bc is None:
                post.append(lambda i, o: nc.vector.tensor_tensor(
                    o, i, gb_ap(ln_idx, "b"), ALU.add))
            elif bc != 0.0:
                post.append(lambda i, o: nc.vector.tensor_scalar_add(
                    o, i, bc))
            if not post:
                # chunked at NCH so the next conv's first-half matmuls (which
                # read padded cols <= 518) start before the tail is written
                nc.vector.tensor_scalar(dst[:, 0:NCH], src[:, 0:NCH], rs[:],
                                        murs[:], op0=ALU.mult,
                                        op1=ALU.subtract)
                nc.vector.tensor_scalar(dst[:, NCH:T], src[:, NCH:T], rs[:],
                                        murs[:], op0=ALU.mult,
                                        op1=ALU.subtract)
            else:
                mids = [work.tile([C, T], BF16, tag="lnmid", bufs=2),
                        work.tile([C, T], BF16, tag="lnmid2", bufs=2)]
                nc.vector.tensor_scalar(mids[0][:], src, rs[:], murs[:],
                                        op0=ALU.mult, op1=ALU.subtract)
                for i, emit in enumerate(post):
                    last = (i == len(post) - 1)
                    emit(mids[i % 2][:], dst if last else mids[(i + 1) % 2][:])

        # ---- x + pos -> xpad (conv0 input); LN0 -> res ----
        res = resp.tile([C, BPC * T], BF16, tag="res")
        for b in range(BPC):
            stats = tiny.tile([C, 4], F32, tag="stats")
            scr = work.tile([C, T], BF16, tag="sqscr")
            for h in range(2):
                sl = slice(h * H, (h + 1) * H)
                nc.vector.scalar_tensor_tensor(
                    ip(b, h * H, H), xin[:, b * T + h * H:b * T + (h + 1) * H],
                    1.0, pos_sb[:, sl],
                    op0=ALU.mult, op1=ALU.add,
                    accum_out=stats[:, 2 * h:2 * h + 1])
                nc.scalar.activation(scr[:, sl], ip(b, h * H, H), AF.Square,
                                     accum_out=stats[:, 2 * h + 1:2 * h + 2])
            emit_ln_tail(0, b, stats, ip(b), res[:, b * T:(b + 1) * T])

        # ---- conv layers ----
        for li in range(NCONV):
            new_res = resp.tile([C, BPC * T], BF16, tag="res")
            for b in range(BPC):
                stats = tiny.tile([C, 4], F32, tag="stats")
                scr = work.tile([C, T], BF16, tag="sqscr")
                nr = new_res[:, b * T:(b + 1) * T]
                for h in range(2):
                    psc = ps_cv.tile([C, H], F32, tag="ps_cv")
                    for d in range(KW):
                        nc.tensor.matmul(
                            psc[:],
                            w2_sb[:, (li * KW + d) * C:(li * KW + d + 1) * C],
                            xpad[:, b * PADT + h * H + d:
                                 b * PADT + h * H + d + H],
                            start=(d == 0), stop=(d == KW - 1))
                    sl = slice(h * H, (h + 1) * H)
                    if flags.b2_zero:
                        nc.vector.scalar_tensor_tensor(
                            nr[:, sl], psc[:], 0.0,
                            res[:, b * T + h * H:b * T + (h + 1) * H],
                            op0=ALU.max, op1=ALU.add,
                            accum_out=stats[:, 2 * h:2 * h + 1])
                    else:
                        relu_t = work.tile([C, T], BF16, tag="relu_t")
                        nc.scalar.activation(relu_t[:, sl], psc[:],
                                             AF.Relu, bias=b2_sb[:, li:li + 1])
                        nc.vector.scalar_tensor_tensor(
                            nr[:, sl], relu_t[:, sl], 1.0,
                            res[:, b * T + h * H:b * T + (h + 1) * H],
                            op0=ALU.mult, op1=ALU.add,
                            accum_out=stats[:, 2 * h:2 * h + 1])
                    nc.scalar.activation(scr[:, sl], nr[:, sl], AF.Square,
                                         accum_out=stats[:, 2 * h + 1:2 * h + 2])
                emit_ln_tail(1 + li, b, stats, nr, ip(b))
            res = new_res

        # ---- attention + LNe + FC, per batch (batches overlap via
        # per-batch PSUM rings) ----
        new_res = resp.tile([C, BPC * T], BF16, tag="res")
        for b in range(BPC):
            pst = f"ps_at{b}"
            stats = tiny.tile([C, 4], F32, tag="stats")
            xa = ip(b)  # [128, 1024] bf16
            # y = M^T x  (scores s^T[tk,tq] = sum_c' x[c',tk] y[c',tq])
            y_sb = work.tile([C, T], BF16, tag="y_sb")
            for h in range(2):
                sl = slice(h * H, (h + 1) * H)
                psy = ps_at.tile([C, H], F32, tag=pst)
                nc.tensor.matmul(psy[:], m_sb, xa[:, sl],
                                 start=True, stop=True)
                nc.vector.tensor_copy(y_sb[:, sl], psy[:])
            # v in [t, d] layout + appended ones column (denominator)
            psv = ps_at.tile([C, H], F32, tag=pst)
            for j in range(8):
                nc.tensor.matmul(psv[:, j * DK:(j + 1) * DK],
                                 xa[:, j * C:(j + 1) * C], wv_sb,
                                 start=True, stop=True)
            vt = work.tile([C, 8, DK + 1], BF16, tag="vt")
            nc.vector.memset(vt[:, :, DK:DK + 1], 1.0)
            nc.vector.tensor_copy(
                vt[:, :, 0:DK],
                psv[:].rearrange("p (j k) -> p j k", k=DK))
            # scores (transposed: [tk, tq]) + exp, per (key chunk, half)
            eT = work.tile([C, 8 * T], BF16, tag="eT", bufs=2)
            for j in range(8):
                for h in range(2):
                    pss = ps_at.tile([C, H], F32, tag=pst)
                    nc.tensor.matmul(pss[:],
                                     xa[:, j * C:(j + 1) * C],
                                     y_sb[:, h * H:(h + 1) * H],
                                     start=True, stop=True)
                    esl = slice(j * T + h * H, j * T + (h + 1) * H)
                    if flags.mask_ones:
                        nc.scalar.activation(eT[:, esl], pss[:], AF.Exp)
                    else:
                        nc.scalar.activation(
                            eT[:, esl], pss[:], AF.Exp,
                            bias=mb_sb[:, b * 8 + j:b * 8 + j + 1])
            # attention-weighted values + denominator (appended ones column)
            psa = ps_av.tile([DK + 1, T], F32, tag="ps_av")
            for h in range(2):
                for j in range(8):
                    nc.tensor.matmul(
                        psa[:, h * H:(h + 1) * H], vt[:, j, :],
                        eT[:, j * T + h * H: j * T + h * H + H],
                        start=(j == 0), stop=(j == 7))
            # Per-half tail: reciprocal of the denominator row on DVE,
            # partition-broadcast on Pool, normalize+cast on DVE, Wo matmul.
            rr = tiny.tile([1, T], F32, tag="rr", bufs=2)
            rrb = work.tile([DK, T], F32, tag="rrb", bufs=2)
            avn = work.tile([DK, T], BF16, tag="avn")
            nr = new_res[:, b * T:(b + 1) * T]
            scr = work.tile([C, T], BF16, tag="sqscr")
            for h in range(2):
                sl = slice(h * H, (h + 1) * H)
                if flags.mask_ones:
                    nc.vector.reciprocal(rr[:, sl], psa[DK:DK + 1, sl])
                else:
                    # guard fully-masked queries (denom==0 -> finite rr; av
                    # rows are 0 so the product stays 0)
                    nc.vector.tensor_scalar_add(rr[:, sl],
                                                psa[DK:DK + 1, sl], 1e-30)
                    nc.vector.reciprocal(rr[:, sl], rr[:, sl])
                nc.gpsimd.partition_broadcast(rrb[:, sl], rr[:, sl],
                                              channels=DK)
                nc.vector.tensor_tensor(avn[:, sl], psa[0:DK, sl],
                                        rrb[:, sl], ALU.mult)
                pso = ps_at.tile([C, H], F32, tag=pst)
                nc.tensor.matmul(pso[:], wo_sb, avn[:, sl],
                                 start=True, stop=True)
                rsl = slice(b * T + h * H, b * T + (h + 1) * H)
                if flags.mask_ones:
                    nc.vector.scalar_tensor_tensor(
                        nr[:, sl], pso[:], 1.0, res[:, rsl],
                        op0=ALU.mult, op1=ALU.add,
                        accum_out=stats[:, 2 * h:2 * h + 1])
                else:
                    # query-mask rows broadcast across partitions (Pool),
                    # applied before the residual add
                    qmb = work.tile([C, T], F32, tag="qmb")
                    att = work.tile([C, T], BF16, tag="att_m")
                    nc.gpsimd.partition_broadcast(
                        qmb[:, sl],
                        qm_sb[:, b * T + h * H:b * T + (h + 1) * H],
                        channels=C)
                    nc.vector.tensor_tensor(att[:, sl], pso[:], qmb[:, sl],
                                            ALU.mult)
                    nc.vector.scalar_tensor_tensor(
                        nr[:, sl], att[:, sl], 1.0, res[:, rsl],
                        op0=ALU.mult, op1=ALU.add,
                        accum_out=stats[:, 2 * h:2 * h + 1])
                # square on DVE here: ACT is saturated by softmax exp
                nc.vector.scalar_tensor_tensor(
                    scr[:, sl], nr[:, sl], 1.0, nr[:, sl],
                    op0=ALU.mult, op1=ALU.mult,
                    accum_out=stats[:, 2 * h + 1:2 * h + 2])
            emit_ln_tail(NLN - 1, b, stats, nr, ip(b))
            # ---- final FC + relu + residual (same-batch PSUM ring) ----
            for h in range(2):
                psf = ps_at.tile([C, H], F32, tag=pst)
                nc.tensor.matmul(psf[:], fcw_sb, ip(b, h * H, H),
                                 start=True, stop=True)
                sl = slice(h * H, (h + 1) * H)
                ob = out_sb[:, b * T + h * H:b * T + (h + 1) * H]
                rb_ = nr[:, sl]
                if flags.fcb_zero:
                    nc.vector.scalar_tensor_tensor(
                        ob, psf[:], 0.0, rb_, op0=ALU.max, op1=ALU.add)
                else:
                    relu_t = work.tile([C, T], BF16, tag="relu_t")
                    nc.scalar.activation(relu_t[:, sl], psf[:], AF.Relu,
                                         bias=fcb_sb[:])
                    nc.vector.tensor_tensor(ob, relu_t[:, sl], rb_, ALU.add)
                (nc.sync if h == 0 else nc.scalar).dma_start(
                    out_d[b][:, sl], ob)

    nc.compile()
    return nc


_CACHE: dict = {}
LAST_RUN: dict = {}   # exposed for test harnesses (nc, in_maps)


def kernel(x, mask, dw_w, dw_b, pw_w, pw_b, norm0_g, norm0_b,
           norms_g, norms_b, norme_g, norme_b,
           Wq, Wk, Wv, Wo, fc_w, fc_b):
    x = np.asarray(x, dtype=np.float32)
    mask = np.asarray(mask, dtype=np.float32)

    # ---- host-side constant folding ----
    w2 = np.empty((C, NCONV, KW, C), dtype=np.float32)
    for i in range(NCONV):
        pwT = np.asarray(pw_w[i], np.float32).T          # [c, o]
        for d in range(KW):
            w2[:, i, d, :] = pwT * np.asarray(dw_w[i][:, d],
                                              np.float32)[:, None]
    w2 = w2.reshape(C, NCONV * KW * C)
    b2 = np.stack([np.asarray(pw_w[i], np.float32)
                   @ np.asarray(dw_b[i], np.float32)
                   + np.asarray(pw_b[i], np.float32)
                   for i in range(NCONV)], axis=1)        # [C, NCONV]
    mqk = (np.asarray(Wq, np.float32) @ np.asarray(Wk, np.float32).T
           / math.sqrt(DK))                               # [C, C]
    wo = np.asarray(Wo, np.float32)
    wo_eff = np.ascontiguousarray(wo[:DK] + wo[DK:])      # [DK, C]
    fcw = np.ascontiguousarray(np.asarray(fc_w, np.float32).T)  # [c, o]
    fcb = np.asarray(fc_b, np.float32).reshape(C, 1)
    pos = _pos_encoding()

    gs = [norm0_g] + [norms_g[i] for i in range(NCONV)] + [norme_g]
    bs = [norm0_b] + [norms_b[i] for i in range(NCONV)] + [norme_b]
    ln_gc = [_uniform_val(np.asarray(g, np.float32)) for g in gs]
    ln_bc = [_uniform_val(np.asarray(bb, np.float32)) for bb in bs]
    flags = _Flags(ln_gc, ln_bc,
                   b2_zero=not b2.any(),
                   fcb_zero=not fcb.any(),
                   mask_ones=bool(np.all(mask == 1.0)))

    key = flags.key()
    if key not in _CACHE:
        _CACHE[key] = _build(flags)
    nc = _CACHE[key]

    wcat = np.zeros((C, 3 * C + DK), np.float32)
    wcat[:, 0:C] = mqk
    wcat[:, C:C + DK] = np.asarray(Wv, np.float32)
    wcat[:, C + DK:2 * C + DK] = fcw
    wcat[0:DK, 2 * C + DK:3 * C + DK] = wo_eff
    base = {"pos": _bf(pos), "w2": _bf(w2), "wcat": _bf(wcat)}
    if not flags.b2_zero:
        base["b2"] = np.ascontiguousarray(b2)
    if not flags.fcb_zero:
        base["fcb"] = np.ascontiguousarray(fcb)
    gb_entries = ([(l, "g") for l in range(NLN) if flags.ln_gc[l] is None]
                  + [(l, "b") for l in range(NLN) if flags.ln_bc[l] is None])
    if gb_entries:
        gb = np.empty((C, len(gb_entries) * T), np.float32)
        for i, (l, kind) in enumerate(gb_entries):
            src = gs[l] if kind == "g" else bs[l]
            gb[:, i * T:(i + 1) * T] = np.asarray(src, np.float32)
        base["gb"] = _bf(gb)

    in_maps = []
    for c in range(NCORES):
        m = dict(base)
        xc = x[c * BPC:(c + 1) * BPC]                     # [BPC, C, T]
        m["x"] = _bf(xc.transpose(1, 0, 2).reshape(C, BPC * T))
        if not flags.mask_ones:
            msk = mask[c * BPC:(c + 1) * BPC]             # [BPC, T]
            mb = np.where(msk == 0.0, np.float32(-1e9), np.float32(0.0))
            # [128, BPC*8]: column (b*8+j) = bias for key chunk j of batch b
            m["mb"] = np.ascontiguousarray(
                mb.reshape(BPC, 8, C).transpose(2, 0, 1).reshape(C, BPC * 8))
            m["qm"] = np.ascontiguousarray(msk)
        in_maps.append(m)

    LAST_RUN["nc"] = nc
    LAST_RUN["in_maps"] = in_maps

    res = run_bass_kernel_spmd(nc, in_maps, list(range(NCORES)))
    out = np.concatenate([np.asarray(r["out"]) for r in res.results], axis=0)
    return out.astype(np.float32)


# revision 30
# speedup vs baseline: 1.2688x; 1.0055x over previous
"""Trainium2 Bass kernel for nn_Encoder_Block (dense transformer encoder block).

Strategy: pure data parallel across 8 NeuronCores (B=16 -> 2 batch elems per
core), all weights replicated.  Entire block computed on-chip per batch elem:
  x + pos -> LN0 (res) -> 4x [dsconv -> relu -> +res -> LN] -> attention
  -> +res -> LNe -> FC -> relu -> +res

Host-side weight folding (constant prep, input-independent layout changes):
  - depthwise(k=7) + pointwise conv folded into 7 full [C,C] matmul taps:
      W2[c,o; tap d] = pw_w[o,c] * dw_w[c,d]    (accumulated in PSUM over d)
  - conv bias folded: b2 = pw_w @ dw_b + pw_b
  - attention score matrix folded: M = Wq @ Wk^T / sqrt(dk), so the scores
    are computed as s^T = x^T (M^T x) with no separate q/k heads on chip
  - both (identical) heads folded: Wo_eff = Wo[:dk] + Wo[dk:]
  - positional-encoding table precomputed (input-independent constant)

Data plane is bf16 (weights, activations, residuals); statistics, PSUM
accumulation and the final output stay fp32.  LayerNorm over (C,T) jointly:
per-partition sums via accum_out on the producing ops, cross-partition
reduction via a single gpsimd partition_all_reduce on the otherwise-idle
Pool engine; rsqrt via exp(-0.5*ln(var+eps)) so the whole kernel stays in
one scalar-engine table set (natural_log_exp_and_others).  Softmax: scores
computed transposed [tk,tq]; exp without max-subtraction (values bounded,
safe); denominator via an appended ones-column in the PV matmul; reciprocal
on DVE, broadcast to partitions via gpsimd partition_broadcast (Pool).

LayerNorm outputs are written in chunks [0:520] / [520:1024] so the next
conv layer's first-half taps (which read padded cols 0..518) start as soon
as the first chunk lands.  PSUM rings: conv accumulators rotate through a
2-deep [C,512] ring; each batch's attention/FC PSUM rotates through its own
2-deep [C,512] ring so the two batches' attention phases overlap.
"""

import sys

sys.path.insert(0, "/opt/trn_rl_repo")

import math

import numpy as np
import ml_dtypes

import concourse.bass as bass
import concourse.bass_isa as bass_isa
import concourse.tile as tile
from concourse import bacc, library_config, mybir
from concourse.bass_utils import run_bass_kernel_spmd

F32 = mybir.dt.float32
BF16 = mybir.dt.bfloat16
AF = mybir.ActivationFunctionType
ALU = mybir.AluOpType

B, C, T = 16, 128, 1024
NCONV, KW = 4, 7
DK = C // 2
NCORES = 8
BPC = B // NCORES          # batch elems per core
EPS = 1e-5
NEL = float(C * T)         # layernorm element count
PADT = T + KW - 1          # 1030: per-batch padded row in xpad
NLN = NCONV + 2            # LN0, 4 conv LNs, LNe
H = 512                    # half-width of T
NCH = 520                  # norm chunk boundary (covers conv h0 tap reads)


def _pos_encoding() -> np.ndarray:
    i = np.arange(C)
    exp = -((i - (i % 2)).astype(np.float32) / np.float32(C))
    freqs = (np.float32(10000.0) ** exp)[:, None].astype(np.float32)
    phases = ((i % 2).astype(np.float32) * np.float32(np.pi / 2))[:, None]
    pos = np.arange(T, dtype=np.float32)[None, :]
    return np.sin(pos * freqs + phases).astype(np.float32)


def _uniform_val(a: np.ndarray):
    """Return the scalar if all elements equal, else None."""
    v = a.flat[0]
    return float(v) if np.all(a == v) else None


def _bf(a: np.ndarray) -> np.ndarray:
    return np.ascontiguousarray(a.astype(ml_dtypes.bfloat16))


class _Flags:
    """Kernel-structure flags derived from host inspection of the inputs."""

    def __init__(self, ln_gc, ln_bc, b2_zero, fcb_zero, mask_ones):
        self.ln_gc = tuple(ln_gc)
        self.ln_bc = tuple(ln_bc)
        self.b2_zero = b2_zero
        self.fcb_zero = fcb_zero
        self.mask_ones = mask_ones

    def key(self):
        return (self.ln_gc, self.ln_bc, self.b2_zero, self.fcb_zero,
                self.mask_ones)


class _Bacc(bacc.Bacc):
    """Bacc with activation-table choice pinned to the one set that covers
    every function this kernel uses (ln/exp/square/copy/relu/identity).

    The stock insertion pass greedily picks the first act_info set containing
    each function, which lands exp/square in `exp_and_others` and ln in
    `natural_log` — ping-ponging ~2 table loads (~5us) per layernorm.  Hiding
    our functions from every other set (ids/positions unchanged) forces all
    loads to `natural_log_exp_and_others`, so exactly one load is emitted.
    """

    _OURS = {AF.Ln, AF.Exp, AF.Square, AF.Copy, AF.Identity, AF.Relu}
    _KEEP = "natural_log_exp_and_others"

    def insert_act_table_loads(self):
        from concourse.bacc import _bass_rust, get_activation_tables
        has_activation = any(
            isinstance(i, mybir.InstActivation)
            for b in self.main_func.blocks
            for i in b.instructions
        )
        if not has_activation:
            return
        tables = [
            (nm, fs if nm == self._KEEP else (fs - self._OURS))
            for nm, fs in get_activation_tables(self.m.arch).items()
        ]
        _bass_rust.insert_act_table_loads(self, tables)


def _build(flags: _Flags):
    nc = _Bacc("TRN2", target_bir_lowering=False, debug=False,
               num_devices=NCORES)

    def dram(name, shape, dtype=F32, kind="ExternalInput"):
        return nc.dram_tensor(name, shape, dtype, kind=kind).ap()

    x_d = dram("x", [C, BPC * T], BF16)       # host pre-transposed [C, b*T]
    pos_d = dram("pos", [C, T], BF16)
    w2_d = dram("w2", [C, NCONV * KW * C], BF16)
    # packed small weights: [mqk | wv | fcw | wo(rows 0:DK)]
    wc_d = dram("wcat", [C, 3 * C + DK], BF16)
    out_d = dram("out", [BPC, C, T], BF16, kind="ExternalOutput")
    if not flags.b2_zero:
        b2_d = dram("b2", [C, NCONV])
    if not flags.fcb_zero:
        fcb_d = dram("fcb", [C, 1])
    gb_entries = ([(l, "g") for l in range(NLN) if flags.ln_gc[l] is None]
                  + [(l, "b") for l in range(NLN) if flags.ln_bc[l] is None])
    if gb_entries:
        gb_d = dram("gb", [C, len(gb_entries) * T], BF16)
    if not flags.mask_ones:
        mb_d = dram("mb", [C, BPC * (T // C)])   # [128, 2*8] key-mask exp bias
        qm_d = dram("qm", [BPC, T])              # query-mask rows

    from contextlib import ExitStack

    with tile.TileContext(nc) as tc, ExitStack() as ctx:
        cst = ctx.enter_context(tc.tile_pool(name="cst", bufs=1))
        big = ctx.enter_context(tc.tile_pool(name="bigbuf", bufs=1))
        resp = ctx.enter_context(tc.tile_pool(name="resp", bufs=2))
        work = ctx.enter_context(tc.tile_pool(name="work", bufs=2))
        tiny = ctx.enter_context(tc.tile_pool(name="tiny", bufs=4))
        ps_cv = ctx.enter_context(
            tc.tile_pool(name="ps_cv", bufs=2, space="PSUM"))
        ps_at = ctx.enter_context(
            tc.tile_pool(name="ps_at", bufs=2, space="PSUM"))
        ps_av = ctx.enter_context(
            tc.tile_pool(name="ps_av", bufs=1, space="PSUM"))

        nc.gpsimd.load_library(library_config.attn)

        # ---- constants / weights in SBUF ----
        const_tiles: dict = {}

        def const_ap(val: float, npart: int = C):
            """[npart,1] fp32 SBUF constant (activation bias operand)."""
            if val == 0.0:
                return 0.0   # pre-registered const AP
            if val not in const_tiles:
                t = cst.tile([C, 1], F32, tag=f"cst{len(const_tiles)}")
                nc.vector.memset(t[:], val)
                const_tiles[val] = t
            return const_tiles[val][0:npart, :]

        F32R = mybir.dt.float32r
        ones_row_f = cst.tile([1, DK], F32, tag="ones_row_f")
        nc.vector.memset(ones_row_f[:], 1.0)
        ones_row_r = cst.tile([1, DK], F32R, tag="ones_row_r")
        nc.scalar.copy(ones_row_r[:], ones_row_f[:])

        # prologue DMAs: x / pos chunked and spread across the three DMA
        # queues (SP + Act HWDGE, gpsimd SWDGE) so LN0 and the first conv's
        # weights arrive as early as possible
        xin = big.tile([C, BPC * T], BF16, tag="xin")
        pos_sb = cst.tile([C, T], BF16, tag="pos")
        w2_sb = cst.tile([C, NCONV * KW * C], BF16, tag="w2")
        wc_sb = cst.tile([C, 3 * C + DK], BF16, tag="wcat")
        LW = KW * C
        nc.sync.dma_start(xin[:], x_d)
        nc.sync.dma_start(w2_sb[:, 0:LW], w2_d[:, 0:LW])
        nc.sync.dma_start(w2_sb[:, LW:NCONV * LW], w2_d[:, LW:NCONV * LW])
        nc.gpsimd.dma_start(pos_sb[:], pos_d)
        nc.scalar.dma_start(wc_sb[:], wc_d)
        m_sb = wc_sb[:, 0:C]
        wv_sb = wc_sb[:, C:C + DK]
        fcw_sb = wc_sb[:, C + DK:2 * C + DK]
        wo_sb = wc_sb[0:DK, 2 * C + DK:3 * C + DK]
        if not flags.b2_zero:
            b2_sb = cst.tile([C, NCONV], F32, tag="b2")
            nc.sync.dma_start(b2_sb[:], b2_d[:])
        if not flags.fcb_zero:
            fcb_sb = cst.tile([C, 1], F32, tag="fcb")
            nc.sync.dma_start(fcb_sb[:], fcb_d[:])
        if gb_entries:
            gb_sb = cst.tile([C, len(gb_entries) * T], BF16, tag="gb")
            nc.sync.dma_start(gb_sb[:], gb_d[:])
            gb_ix = {e: i for i, e in enumerate(gb_entries)}

            def gb_ap(l, kind):
                i0 = gb_ix[(l, kind)] * T
                return gb_sb[:, i0:i0 + T]
        if not flags.mask_ones:
            mb_sb = cst.tile([C, BPC * (T // C)], F32, tag="mb")
            nc.sync.dma_start(mb_sb[:], mb_d[:])
            qm_sb = cst.tile([1, BPC * T], F32, tag="qm")
            for b in range(BPC):
                nc.sync.dma_start(qm_sb[:, b * T:(b + 1) * T], qm_d[b:b + 1, :])

        # conv/attention input activations, zero-padded per batch elem:
        # [0^3 | x_b (1024) | 0^3] so every conv tap is a full N=512 matmul.
        xpad = big.tile([C, BPC * PADT], BF16, tag="xpad")
        for b in range(BPC):
            nc.vector.memset(xpad[:, b * PADT:b * PADT + 3], 0.0)
            nc.vector.memset(xpad[:, b * PADT + 3 + T:(b + 1) * PADT], 0.0)

        out_sb = big.tile([C, BPC * T], BF16, tag="out_sb")

        def ip(b, off=0, n=T):
            """AP of the xpad interior for batch b (bf16)."""
            return xpad[:, b * PADT + 3 + off: b * PADT + 3 + off + n]

        def emit_ln_tail(ln_idx, b, stats, src, dst):
            """Finish LayerNorm over (C,T) for one batch elem.

            stats: [C,4] fp32 tile holding per-partition partial sums
            (S1_h0, S2_h0, S1_h1, S2_h1) from the producing ops' accum_out.
            src: bf16 [C,T] AP (the residual-stream value to normalize);
            dst: bf16 [C,T] AP (LN output, feeds matmuls).
            """
            ctx2 = ExitStack()
            ctx2.enter_context(tc.high_priority())
            red = tiny.tile([C, 4], F32, tag="red")
            nc.gpsimd.partition_all_reduce(red[:], stats[:], channels=C,
                                           reduce_op=bass_isa.ReduceOp.add)
            # mom = [U, V] = total [sum, sum-of-squares] (halves combined)
            mom = tiny.tile([C, 2], F32, tag="mom")
            nc.vector.tensor_tensor(mom[:], red[:, 0:2], red[:, 2:4], ALU.add)
            # musq = U^2/N ; vare = (V + N*eps) - U^2/N = N*(var+eps)
            musq = tiny.tile([C, 1], F32, tag="musq")
            nc.vector.scalar_tensor_tensor(musq[:], mom[:, 0:1], 1.0 / NEL,
                                           mom[:, 0:1],
                                           op0=ALU.mult, op1=ALU.mult)
            vare = tiny.tile([C, 1], F32, tag="vare")
            nc.vector.scalar_tensor_tensor(vare[:], mom[:, 1:2], NEL * EPS,
                                           musq[:],
                                           op0=ALU.add, op1=ALU.subtract)
            # rs = gc / sqrt(var+eps) = exp(-0.5*ln(N*(var+eps)) + 0.5*ln N
            #      + ln(gc)); stays in the natural_log_exp ACT table set.
            gc, bc = flags.ln_gc[ln_idx], flags.ln_bc[ln_idx]
            lnv = tiny.tile([C, 1], F32, tag="lnv")
            nc.scalar.activation(lnv[:], vare[:], AF.Ln)
            rs = tiny.tile([C, 1], F32, tag="rs")
            expb = (0.5 * math.log(NEL)
                    + (math.log(gc) if (gc is not None and gc > 0.0) else 0.0))
            nc.scalar.activation(rs[:], lnv[:], AF.Exp, scale=-0.5,
                                 bias=const_ap(expb))
            if gc is not None and gc <= 0.0 and gc != 1.0:
                rs2 = tiny.tile([C, 1], F32, tag="rs2")
                nc.vector.tensor_scalar_mul(rs2[:], rs[:], gc)
                rs = rs2
            # murs = (U/N)*rs so normalize is x*rs - murs (both 4x-capable)
            murs = tiny.tile([C, 1], F32, tag="murs")
            nc.vector.scalar_tensor_tensor(murs[:], mom[:, 0:1], 1.0 / NEL,
                                           rs[:],
                                           op0=ALU.mult, op1=ALU.mult)
            post = []
            if gc is None:
                post.append(lambda i, o: nc.vector.tensor_tensor(
                    o, i, gb_ap(ln_idx, "g"), ALU.mult))
            if bc is None:
                post.append(lambda i, o: nc.vector.tensor_tensor(
                    o, i, gb_ap(ln_idx, "b"), ALU.add))
            elif bc != 0.0:
                post.append(lambda i, o: nc.vector.tensor_scalar_add(
                    o, i, bc))
            if not post:
                # chunked at NCH so the next conv's first-half matmuls (which
                # read padded cols <= 518) start before the tail is written
                nc.vector.tensor_scalar(dst[:, 0:NCH], src[:, 0:NCH], rs[:],
                                        murs[:], op0=ALU.mult,
                                        op1=ALU.subtract)
                nc.vector.tensor_scalar(dst[:, NCH:T], src[:, NCH:T], rs[:],
                                        murs[:], op0=ALU.mult,
                                        op1=ALU.subtract)
                ctx2.close()
            else:
                ctx2.close()
                mids = [work.tile([C, T], BF16, tag="lnmid", bufs=2),
                        work.tile([C, T], BF16, tag="lnmid2", bufs=2)]
                nc.vector.tensor_scalar(mids[0][:], src, rs[:], murs[:],
                                        op0=ALU.mult, op1=ALU.subtract)
                for i, emit in enumerate(post):
                    last = (i == len(post) - 1)
                    emit(mids[i % 2][:], dst if last else mids[(i + 1) % 2][:])

        # ---- x + pos -> xpad (conv0 input); LN0 -> res ----
        res = resp.tile([C, BPC * T], BF16, tag="res")
        for b in range(BPC):
            stats = tiny.tile([C, 4], F32, tag="stats")
            scr = work.tile([C, T], BF16, tag="sqscr")
            for h in range(2):
                sl = slice(h * H, (h + 1) * H)
                nc.vector.scalar_tensor_tensor(
                    ip(b, h * H, H), xin[:, b * T + h * H:b * T + (h + 1) * H],
                    1.0, pos_sb[:, sl],
                    op0=ALU.mult, op1=ALU.add,
                    accum_out=stats[:, 2 * h:2 * h + 1])
                # h1 square on DVE right after its producer (no engine hop on
                # the stats critical path); h0 square on ACT in parallel
                if h == 0:
                    nc.scalar.activation(scr[:, sl], ip(b, h * H, H),
                                         AF.Square,
                                         accum_out=stats[:, 2 * h + 1:2 * h + 2])
                else:
                    nc.vector.scalar_tensor_tensor(
                        scr[:, sl], ip(b, h * H, H), 1.0, ip(b, h * H, H),
                        op0=ALU.mult, op1=ALU.mult,
                        accum_out=stats[:, 2 * h + 1:2 * h + 2])
            emit_ln_tail(0, b, stats, ip(b), res[:, b * T:(b + 1) * T])

        # ---- conv layers ----
        for li in range(NCONV):
            new_res = resp.tile([C, BPC * T], BF16, tag="res")
            for b in range(BPC):
                stats = tiny.tile([C, 4], F32, tag="stats")
                scr = work.tile([C, T], BF16, tag="sqscr")
                nr = new_res[:, b * T:(b + 1) * T]
                for h in range(2):
                    psc = ps_cv.tile([C, H], F32, tag="ps_cv")
                    for d in range(KW):
                        nc.tensor.matmul(
                            psc[:],
                            w2_sb[:, (li * KW + d) * C:(li * KW + d + 1) * C],
                            xpad[:, b * PADT + h * H + d:
                                 b * PADT + h * H + d + H],
                            start=(d == 0), stop=(d == KW - 1))
                    sl = slice(h * H, (h + 1) * H)
                    if flags.b2_zero:
                        nc.vector.scalar_tensor_tensor(
                            nr[:, sl], psc[:], 0.0,
                            res[:, b * T + h * H:b * T + (h + 1) * H],
                            op0=ALU.max, op1=ALU.add,
                            accum_out=stats[:, 2 * h:2 * h + 1])
                    else:
                        relu_t = work.tile([C, T], BF16, tag="relu_t")
                        nc.scalar.activation(relu_t[:, sl], psc[:],
                                             AF.Relu, bias=b2_sb[:, li:li + 1])
                        nc.vector.scalar_tensor_tensor(
                            nr[:, sl], relu_t[:, sl], 1.0,
                            res[:, b * T + h * H:b * T + (h + 1) * H],
                            op0=ALU.mult, op1=ALU.add,
                            accum_out=stats[:, 2 * h:2 * h + 1])
                    if h == 0:
                        nc.scalar.activation(
                            scr[:, sl], nr[:, sl], AF.Square,
                            accum_out=stats[:, 2 * h + 1:2 * h + 2])
                    else:
                        nc.vector.scalar_tensor_tensor(
                            scr[:, sl], nr[:, sl], 1.0, nr[:, sl],
                            op0=ALU.mult, op1=ALU.mult,
                            accum_out=stats[:, 2 * h + 1:2 * h + 2])
                emit_ln_tail(1 + li, b, stats, nr, ip(b))
            res = new_res

        # ---- attention + LNe + FC, per batch (batches overlap via
        # per-batch PSUM rings) ----
        new_res = resp.tile([C, BPC * T], BF16, tag="res")
        for b in range(BPC):
            pst = f"ps_at{b}"
            stats = tiny.tile([C, 4], F32, tag="stats")
            xa = ip(b)  # [128, 1024] bf16
            # y = M^T x  (scores s^T[tk,tq] = sum_c' x[c',tk] y[c',tq])
            y_sb = work.tile([C, T], BF16, tag="y_sb")
            for h in range(2):
                sl = slice(h * H, (h + 1) * H)
                psy = ps_at.tile([C, H], F32, tag=pst)
                nc.tensor.matmul(psy[:], m_sb, xa[:, sl],
                                 start=True, stop=True)
                nc.vector.tensor_copy(y_sb[:, sl], psy[:])
            # v in [t, d] layout + appended ones column (denominator)
            psv = ps_at.tile([C, H], F32, tag=pst)
            for j in range(8):
                nc.tensor.matmul(psv[:, j * DK:(j + 1) * DK],
                                 xa[:, j * C:(j + 1) * C], wv_sb,
                                 start=True, stop=True)
            vt = work.tile([C, 8, DK + 1], BF16, tag="vt")
            nc.vector.memset(vt[:, :, DK:DK + 1], 1.0)
            nc.vector.tensor_copy(
                vt[:, :, 0:DK],
                psv[:].rearrange("p (j k) -> p j k", k=DK))
            # scores (transposed: [tk, tq]) + exp, per (key chunk, half)
            eT = work.tile([C, 8 * T], BF16, tag="eT", bufs=2)
            for j in range(8):
                for h in range(2):
                    pss = ps_at.tile([C, H], F32, tag=pst)
                    nc.tensor.matmul(pss[:],
                                     xa[:, j * C:(j + 1) * C],
                                     y_sb[:, h * H:(h + 1) * H],
                                     start=True, stop=True)
                    esl = slice(j * T + h * H, j * T + (h + 1) * H)
                    if flags.mask_ones:
                        nc.scalar.activation(eT[:, esl], pss[:], AF.Exp)
                    else:
                        nc.scalar.activation(
                            eT[:, esl], pss[:], AF.Exp,
                            bias=mb_sb[:, b * 8 + j:b * 8 + j + 1])
            # attention-weighted values + denominator (appended ones column)
            psa = ps_av.tile([DK + 1, T], F32, tag="ps_av")
            for h in range(2):
                for j in range(8):
                    nc.tensor.matmul(
                        psa[:, h * H:(h + 1) * H], vt[:, j, :],
                        eT[:, j * T + h * H: j * T + h * H + H],
                        start=(j == 0), stop=(j == 7))
            # Per-half tail: reciprocal of the denominator row on DVE,
            # broadcast to partitions via a PE ones-matmul (PE and ACT are
            # idle in the tail; DVE is the contended engine), av copy on ACT,
            # normalize+cast on DVE, Wo matmul.
            rr = tiny.tile([1, T], F32R, tag="rr", bufs=2)
            av = work.tile([DK, T], BF16, tag="av")
            rrb = work.tile([DK, T], F32, tag="rrb", bufs=2)
            avn = work.tile([DK, T], BF16, tag="avn")
            nr = new_res[:, b * T:(b + 1) * T]
            scr = work.tile([C, T], BF16, tag="sqscr")
            # b0's tail overlaps b1's softmax exps (ACT saturated), so it
            # normalizes via Pool partition_broadcast + DVE; the last batch's
            # tail has ACT and PE idle, so it uses an ACT av-copy and a PE
            # ones-matmul broadcast instead.
            last = (b == BPC - 1)
            for h in range(2):
                sl = slice(h * H, (h + 1) * H)
                with nc.allow_low_precision("softmax denom reciprocal; "
                                             "f32r tag only changes mm mode"):
                    if flags.mask_ones:
                        nc.vector.reciprocal(rr[:, sl], psa[DK:DK + 1, sl])
                    else:
                        # guard fully-masked queries (denom==0 -> finite rr;
                        # av rows are 0 so the product stays 0)
                        nc.vector.tensor_scalar_add(rr[:, sl],
                                                    psa[DK:DK + 1, sl], 1e-30)
                        nc.vector.reciprocal(rr[:, sl], rr[:, sl])
                if last:
                    nc.scalar.copy(av[:, sl], psa[0:DK, sl])
                    psr = ps_at.tile([C, H], F32, tag=pst)
                    nc.tensor.matmul(psr[0:DK, :], ones_row_r[:], rr[:, sl],
                                     start=True, stop=True)
                    nc.vector.tensor_tensor(avn[:, sl], av[:, sl],
                                            psr[0:DK, :], ALU.mult)
                else:
                    nc.gpsimd.partition_broadcast(
                        rrb[:, sl], rr[:, sl].bitcast(F32), channels=DK)
                    nc.vector.tensor_tensor(avn[:, sl], psa[0:DK, sl],
                                            rrb[:, sl], ALU.mult)
                pso = ps_at.tile([C, H], F32, tag=pst)
                nc.tensor.matmul(pso[:], wo_sb, avn[:, sl],
                                 start=True, stop=True)
                rsl = slice(b * T + h * H, b * T + (h + 1) * H)
                if flags.mask_ones:
                    nc.vector.scalar_tensor_tensor(
                        nr[:, sl], pso[:], 1.0, res[:, rsl],
                        op0=ALU.mult, op1=ALU.add,
                        accum_out=stats[:, 2 * h:2 * h + 1])
                else:
                    # query-mask rows broadcast across partitions (Pool),
                    # applied before the residual add
                    qmb = work.tile([C, T], F32, tag="qmb")
                    att = work.tile([C, T], BF16, tag="att_m")
                    nc.gpsimd.partition_broadcast(
                        qmb[:, sl],
                        qm_sb[:, b * T + h * H:b * T + (h + 1) * H],
                        channels=C)
                    nc.vector.tensor_tensor(att[:, sl], pso[:], qmb[:, sl],
                                            ALU.mult)
                    nc.vector.scalar_tensor_tensor(
                        nr[:, sl], att[:, sl], 1.0, res[:, rsl],
                        op0=ALU.mult, op1=ALU.add,
                        accum_out=stats[:, 2 * h:2 * h + 1])
                if last:
                    nc.scalar.activation(
                        scr[:, sl], nr[:, sl], AF.Square,
                        accum_out=stats[:, 2 * h + 1:2 * h + 2])
                else:
                    nc.vector.scalar_tensor_tensor(
                        scr[:, sl], nr[:, sl], 1.0, nr[:, sl],
                        op0=ALU.mult, op1=ALU.mult,
                        accum_out=stats[:, 2 * h + 1:2 * h + 2])
            emit_ln_tail(NLN - 1, b, stats, nr, ip(b))
            # ---- final FC + relu + residual (same-batch PSUM ring) ----
            for h in range(2):
                psf = ps_at.tile([C, H], F32, tag=pst)
                nc.tensor.matmul(psf[:], fcw_sb, ip(b, h * H, H),
                                 start=True, stop=True)
                sl = slice(h * H, (h + 1) * H)
                ob = out_sb[:, b * T + h * H:b * T + (h + 1) * H]
                rb_ = nr[:, sl]
                if flags.fcb_zero:
                    nc.vector.scalar_tensor_tensor(
                        ob, psf[:], 0.0, rb_, op0=ALU.max, op1=ALU.add)
                else:
                    relu_t = work.tile([C, T], BF16, tag="relu_t")
                    nc.scalar.activation(relu_t[:, sl], psf[:], AF.Relu,
                                         bias=fcb_sb[:])
                    nc.vector.tensor_tensor(ob, relu_t[:, sl], rb_, ALU.add)
                (nc.sync if h == 0 else nc.scalar).dma_start(
                    out_d[b][:, sl], ob)

    nc.compile()
    return nc


_CACHE: dict = {}
LAST_RUN: dict = {}   # exposed for test harnesses (nc, in_maps)


def kernel(x, mask, dw_w, dw_b, pw_w, pw_b, norm0_g, norm0_b,
           norms_g, norms_b, norme_g, norme_b,
           Wq, Wk, Wv, Wo, fc_w, fc_b):
    x = np.asarray(x, dtype=np.float32)
    mask = np.asarray(mask, dtype=np.float32)

    # ---- host-side constant folding ----
    w2 = np.empty((C, NCONV, KW, C), dtype=np.float32)
    for i in range(NCONV):
        pwT = np.asarray(pw_w[i], np.float32).T          # [c, o]
        for d in range(KW):
            w2[:, i, d, :] = pwT * np.asarray(dw_w[i][:, d],
                                              np.float32)[:, None]
    w2 = w2.reshape(C, NCONV * KW * C)
    b2 = np.stack([np.asarray(pw_w[i], np.float32)
                   @ np.asarray(dw_b[i], np.float32)
                   + np.asarray(pw_b[i], np.float32)
                   for i in range(NCONV)], axis=1)        # [C, NCONV]
    mqk = (np.asarray(Wq, np.float32) @ np.asarray(Wk, np.float32).T
           / math.sqrt(DK))                               # [C, C]
    wo = np.asarray(Wo, np.float32)
    wo_eff = np.ascontiguousarray(wo[:DK] + wo[DK:])      # [DK, C]
    fcw = np.ascontiguousarray(np.asarray(fc_w, np.float32).T)  # [c, o]
    fcb = np.asarray(fc_b, np.float32).reshape(C, 1)
    pos = _pos_encoding()

    gs = [norm0_g] + [norms_g[i] for i in range(NCONV)] + [norme_g]
    bs = [norm0_b] + [norms_b[i] for i in range(NCONV)] + [norme_b]
    ln_gc = [_uniform_val(np.asarray(g, np.float32)) for g in gs]
    ln_bc = [_uniform_val(np.asarray(bb, np.float32)) for bb in bs]
    flags = _Flags(ln_gc, ln_bc,
                   b2_zero=not b2.any(),
                   fcb_zero=not fcb.any(),
                   mask_ones=bool(np.all(mask == 1.0)))

    key = flags.key()
    if key not in _CACHE:
        _CACHE[key] = _build(flags)
    nc = _CACHE[key]

    wcat = np.zeros((C, 3 * C + DK), np.float32)
    wcat[:, 0:C] = mqk
    wcat[:, C:C + DK] = np.asarray(Wv, np.float32)
    wcat[:, C + DK:2 * C + DK] = fcw
    wcat[0:DK, 2 * C + DK:3 * C + DK] = wo_eff
    base = {"pos": _bf(pos), "w2": _bf(w2), "wcat": _bf(wcat)}
    if not flags.b2_zero:
        base["b2"] = np.ascontiguousarray(b2)
    if not flags.fcb_zero:
        base["fcb"] = np.ascontiguousarray(fcb)
    gb_entries = ([(l, "g") for l in range(NLN) if flags.ln_gc[l] is None]
                  + [(l, "b") for l in range(NLN) if flags.ln_bc[l] is None])
    if gb_entries:
        gb = np.empty((C, len(gb_entries) * T), np.float32)
        for i, (l, kind) in enumerate(gb_entries):
            src = gs[l] if kind == "g" else bs[l]
            gb[:, i * T:(i + 1) * T] = np.asarray(src, np.float32)
        base["gb"] = _bf(gb)

    in_maps = []
    for c in range(NCORES):
        m = dict(base)
        xc = x[c * BPC:(c + 1) * BPC]                     # [BPC, C, T]
        m["x"] = _bf(xc.transpose(1, 0, 2).reshape(C, BPC * T))
        if not flags.mask_ones:
            msk = mask[c * BPC:(c + 1) * BPC]             # [BPC, T]
            mb = np.where(msk == 0.0, np.float32(-1e9), np.float32(0.0))
            # [128, BPC*8]: column (b*8+j) = bias for key chunk j of batch b
            m["mb"] = np.ascontiguousarray(
                mb.reshape(BPC, 8, C).transpose(2, 0, 1).reshape(C, BPC * 8))
            m["qm"] = np.ascontiguousarray(msk)
        in_maps.append(m)

    LAST_RUN["nc"] = nc
    LAST_RUN["in_maps"] = in_maps

    res = run_bass_kernel_spmd(nc, in_maps, list(range(NCORES)))
    out = np.concatenate([np.asarray(r["out"]) for r in res.results], axis=0)
    return out.astype(np.float32)


# revision 33
# speedup vs baseline: 1.3264x; 1.0454x over previous
"""Trainium2 Bass kernel for nn_Encoder_Block (dense transformer encoder block).

Strategy: pure data parallel across 8 NeuronCores (B=16 -> 2 batch elems per
core), all weights replicated.  Entire block computed on-chip per batch elem:
  x + pos -> LN0 (res) -> 4x [dsconv -> relu -> +res -> LN] -> attention
  -> +res -> LNe -> FC -> relu -> +res

Host-side weight folding (constant prep, input-independent layout changes):
  - depthwise(k=7) + pointwise conv folded into 7 full [C,C] matmul taps:
      W2[c,o; tap d] = pw_w[o,c] * dw_w[c,d]    (accumulated in PSUM over d)
  - conv bias folded: b2 = pw_w @ dw_b + pw_b
  - attention score matrix folded: M = Wq @ Wk^T / sqrt(dk), so the scores
    are computed as s^T = x^T (M^T x) with no separate q/k heads on chip
  - both (identical) heads folded: Wo_eff = Wo[:dk] + Wo[dk:]
  - positional-encoding table precomputed (input-independent constant)

Data plane is bf16 (weights, activations, residuals); statistics, PSUM
accumulation and the final output stay fp32.  LayerNorm over (C,T) jointly:
per-partition sums via accum_out on the producing ops, cross-partition
reduction via a single gpsimd partition_all_reduce on the otherwise-idle
Pool engine; rsqrt via exp(-0.5*ln(var+eps)) so the whole kernel stays in
one scalar-engine table set (natural_log_exp_and_others).  Softmax: scores
computed transposed [tk,tq]; exp without max-subtraction (values bounded,
safe); denominator via an appended ones-column in the PV matmul; reciprocal
on DVE, broadcast to partitions via gpsimd partition_broadcast (Pool).

LayerNorm outputs are written in chunks [0:520] / [520:1024] so the next
conv layer's first-half taps (which read padded cols 0..518) start as soon
as the first chunk lands.  PSUM rings: conv accumulators rotate through a
2-deep [C,512] ring; each batch's attention/FC PSUM rotates through its own
2-deep [C,512] ring so the two batches' attention phases overlap.
"""

import sys

sys.path.insert(0, "/opt/trn_rl_repo")

import math

import numpy as np
import ml_dtypes

import concourse.bass as bass
import concourse.bass_isa as bass_isa
import concourse.tile as tile
from concourse import bacc, library_config, mybir
from concourse.bass_utils import run_bass_kernel_spmd

F32 = mybir.dt.float32
BF16 = mybir.dt.bfloat16
AF = mybir.ActivationFunctionType
ALU = mybir.AluOpType

B, C, T = 16, 128, 1024
NCONV, KW = 4, 7
DK = C // 2
NCORES = 8
BPC = B // NCORES          # batch elems per core
EPS = 1e-5
NEL = float(C * T)         # layernorm element count
PADT = T + KW - 1          # 1030: per-batch padded row in xpad
NLN = NCONV + 2            # LN0, 4 conv LNs, LNe
H = 512                    # half-width of T
NCH = 520                  # norm chunk boundary (covers conv h0 tap reads)


def _pos_encoding() -> np.ndarray:
    i = np.arange(C)
    exp = -((i - (i % 2)).astype(np.float32) / np.float32(C))
    freqs = (np.float32(10000.0) ** exp)[:, None].astype(np.float32)
    phases = ((i % 2).astype(np.float32) * np.float32(np.pi / 2))[:, None]
    pos = np.arange(T, dtype=np.float32)[None, :]
    return np.sin(pos * freqs + phases).astype(np.float32)


def _uniform_val(a: np.ndarray):
    """Return the scalar if all elements equal, else None."""
    v = a.flat[0]
    return float(v) if np.all(a == v) else None


def _bf(a: np.ndarray) -> np.ndarray:
    return np.ascontiguousarray(a.astype(ml_dtypes.bfloat16))


class _Flags:
    """Kernel-structure flags derived from host inspection of the inputs."""

    def __init__(self, ln_gc, ln_bc, b2_zero, fcb_zero, mask_ones):
        self.ln_gc = tuple(ln_gc)
        self.ln_bc = tuple(ln_bc)
        self.b2_zero = b2_zero
        self.fcb_zero = fcb_zero
        self.mask_ones = mask_ones

    def key(self):
        return (self.ln_gc, self.ln_bc, self.b2_zero, self.fcb_zero,
                self.mask_ones)


class _Bacc(bacc.Bacc):
    """Bacc with activation-table choice pinned to the one set that covers
    every function this kernel uses (ln/exp/square/copy/relu/identity).

    The stock insertion pass greedily picks the first act_info set containing
    each function, which lands exp/square in `exp_and_others` and ln in
    `natural_log` — ping-ponging ~2 table loads (~5us) per layernorm.  Hiding
    our functions from every other set (ids/positions unchanged) forces all
    loads to `natural_log_exp_and_others`, so exactly one load is emitted.
    """

    _OURS = {AF.Ln, AF.Exp, AF.Square, AF.Copy, AF.Identity, AF.Relu}
    _KEEP = "natural_log_exp_and_others"

    def insert_act_table_loads(self):
        from concourse.bacc import _bass_rust, get_activation_tables
        has_activation = any(
            isinstance(i, mybir.InstActivation)
            for b in self.main_func.blocks
            for i in b.instructions
        )
        if not has_activation:
            return
        tables = [
            (nm, fs if nm == self._KEEP else (fs - self._OURS))
            for nm, fs in get_activation_tables(self.m.arch).items()
        ]
        _bass_rust.insert_act_table_loads(self, tables)


def _build(flags: _Flags):
    nc = _Bacc("TRN2", target_bir_lowering=False, debug=False,
               num_devices=NCORES)

    def dram(name, shape, dtype=F32, kind="ExternalInput"):
        return nc.dram_tensor(name, shape, dtype, kind=kind).ap()

    x_d = dram("x", [C, BPC * T], BF16)       # host pre-transposed [C, b*T]
    pos_d = dram("pos", [C, T], BF16)
    w2_d = dram("w2", [C, NCONV * KW * C], BF16)
    # packed small weights: [mqk | wv | fcw | wo(rows 0:DK)]
    wc_d = dram("wcat", [C, 3 * C + DK], BF16)
    out_d = dram("out", [BPC, C, T], BF16, kind="ExternalOutput")
    if not flags.b2_zero:
        b2_d = dram("b2", [C, NCONV])
    if not flags.fcb_zero:
        fcb_d = dram("fcb", [C, 1])
    gb_entries = ([(l, "g") for l in range(NLN) if flags.ln_gc[l] is None]
                  + [(l, "b") for l in range(NLN) if flags.ln_bc[l] is None])
    if gb_entries:
        gb_d = dram("gb", [C, len(gb_entries) * T], BF16)
    if not flags.mask_ones:
        mb_d = dram("mb", [C, BPC * (T // C)])   # [128, 2*8] key-mask exp bias
        qm_d = dram("qm", [BPC, T])              # query-mask rows

    from contextlib import ExitStack

    with tile.TileContext(nc) as tc, ExitStack() as ctx:
        cst = ctx.enter_context(tc.tile_pool(name="cst", bufs=1))
        big = ctx.enter_context(tc.tile_pool(name="bigbuf", bufs=1))
        resp = ctx.enter_context(tc.tile_pool(name="resp", bufs=2))
        work = ctx.enter_context(tc.tile_pool(name="work", bufs=2))
        tiny = ctx.enter_context(tc.tile_pool(name="tiny", bufs=4))
        ps_cv = ctx.enter_context(
            tc.tile_pool(name="ps_cv", bufs=2, space="PSUM"))
        ps_at = ctx.enter_context(
            tc.tile_pool(name="ps_at", bufs=2, space="PSUM"))
        ps_av = ctx.enter_context(
            tc.tile_pool(name="ps_av", bufs=1, space="PSUM"))

        nc.gpsimd.load_library(library_config.attn)

        # ---- constants / weights in SBUF ----
        const_tiles: dict = {}

        def const_ap(val: float, npart: int = C):
            """[npart,1] fp32 SBUF constant (activation bias operand)."""
            if val == 0.0:
                return 0.0   # pre-registered const AP
            if val not in const_tiles:
                t = cst.tile([C, 1], F32, tag=f"cst{len(const_tiles)}")
                nc.vector.memset(t[:], val)
                const_tiles[val] = t
            return const_tiles[val][0:npart, :]

        F32R = mybir.dt.float32r
        ones_row_f = cst.tile([1, DK], F32, tag="ones_row_f")
        nc.vector.memset(ones_row_f[:], 1.0)
        ones_row_r = cst.tile([1, DK], F32R, tag="ones_row_r")
        nc.scalar.copy(ones_row_r[:], ones_row_f[:])
        cNN = cst.tile([C, 2], F32, tag="cNN")      # [1/N, 1/N]
        nc.vector.memset(cNN[:], 1.0 / NEL)
        cNe = cst.tile([C, 1], F32, tag="cNe")      # N*eps
        nc.vector.memset(cNe[:], NEL * EPS)

        # prologue DMAs: x / pos chunked and spread across the three DMA
        # queues (SP + Act HWDGE, gpsimd SWDGE) so LN0 and the first conv's
        # weights arrive as early as possible
        xin = big.tile([C, BPC * T], BF16, tag="xin")
        pos_sb = cst.tile([C, T], BF16, tag="pos")
        w2_sb = cst.tile([C, NCONV * KW * C], BF16, tag="w2")
        wc_sb = cst.tile([C, 3 * C + DK], BF16, tag="wcat")
        LW = KW * C
        # ordered so the LN0(b0) + conv0 critical pieces transfer first on
        # the (serial) DMA engines: x(b0) halves, pos, conv0 weights
        nc.sync.dma_start(xin[:, 0:H], x_d[:, 0:H])
        nc.sync.dma_start(w2_sb[:, 0:LW], w2_d[:, 0:LW])
        nc.sync.dma_start(xin[:, H:T], x_d[:, H:T])
        nc.sync.dma_start(xin[:, T:2 * T], x_d[:, T:2 * T])
        nc.sync.dma_start(w2_sb[:, LW:NCONV * LW], w2_d[:, LW:NCONV * LW])
        nc.gpsimd.dma_start(pos_sb[:, 0:H], pos_d[:, 0:H])
        nc.gpsimd.dma_start(pos_sb[:, H:T], pos_d[:, H:T])
        nc.gpsimd.dma_start(wc_sb[:], wc_d)
        m_sb = wc_sb[:, 0:C]
        wv_sb = wc_sb[:, C:C + DK]
        fcw_sb = wc_sb[:, C + DK:2 * C + DK]
        wo_sb = wc_sb[0:DK, 2 * C + DK:3 * C + DK]
        if not flags.b2_zero:
            b2_sb = cst.tile([C, NCONV], F32, tag="b2")
            nc.sync.dma_start(b2_sb[:], b2_d[:])
        if not flags.fcb_zero:
            fcb_sb = cst.tile([C, 1], F32, tag="fcb")
            nc.sync.dma_start(fcb_sb[:], fcb_d[:])
        if gb_entries:
            gb_sb = cst.tile([C, len(gb_entries) * T], BF16, tag="gb")
            nc.sync.dma_start(gb_sb[:], gb_d[:])
            gb_ix = {e: i for i, e in enumerate(gb_entries)}

            def gb_ap(l, kind):
                i0 = gb_ix[(l, kind)] * T
                return gb_sb[:, i0:i0 + T]
        if not flags.mask_ones:
            mb_sb = cst.tile([C, BPC * (T // C)], F32, tag="mb")
            nc.sync.dma_start(mb_sb[:], mb_d[:])
            qm_sb = cst.tile([1, BPC * T], F32, tag="qm")
            for b in range(BPC):
                nc.sync.dma_start(qm_sb[:, b * T:(b + 1) * T], qm_d[b:b + 1, :])

        # conv/attention input activations, zero-padded per batch elem:
        # [0^3 | x_b (1024) | 0^3] so every conv tap is a full N=512 matmul.
        xpad = big.tile([C, BPC * PADT], BF16, tag="xpad")
        for b in range(BPC):
            nc.vector.memset(xpad[:, b * PADT:b * PADT + 3], 0.0)
            nc.vector.memset(xpad[:, b * PADT + 3 + T:(b + 1) * PADT], 0.0)

        out_sb = big.tile([C, BPC * T], BF16, tag="out_sb")

        def ip(b, off=0, n=T):
            """AP of the xpad interior for batch b (bf16)."""
            return xpad[:, b * PADT + 3 + off: b * PADT + 3 + off + n]

        def emit_ln_tail(ln_idx, b, stats, src, dst):
            """Finish LayerNorm over (C,T) for one batch elem.

            stats: [C,4] fp32 tile holding per-partition partial sums
            (S1_h0, S2_h0, S1_h1, S2_h1) from the producing ops' accum_out.
            src: bf16 [C,T] AP (the residual-stream value to normalize);
            dst: bf16 [C,T] AP (LN output, feeds matmuls).
            """
            ctx2 = ExitStack()
            ctx2.enter_context(tc.high_priority())
            red = tiny.tile([C, 4], F32, tag="red")
            nc.gpsimd.partition_all_reduce(red[:], stats[:], channels=C,
                                           reduce_op=bass_isa.ReduceOp.add)
            # mom = [U, V] = total [sum, sum-of-squares] (halves combined);
            # the small stat ops run on Pool (tensor_tensor only), which is
            # otherwise idle, so they never queue behind bulk DVE/ACT ops
            mom = tiny.tile([C, 2], F32, tag="mom")
            nc.gpsimd.tensor_tensor(mom[:], red[:, 0:2], red[:, 2:4], ALU.add)
            momn = tiny.tile([C, 2], F32, tag="momn")
            nc.gpsimd.tensor_tensor(momn[:], mom[:], cNN[:], ALU.mult)
            # musq = U^2/N ; vare = (V + N*eps) - U^2/N = N*(var+eps)
            musq = tiny.tile([C, 1], F32, tag="musq")
            nc.gpsimd.tensor_tensor(musq[:], momn[:, 0:1], mom[:, 0:1],
                                    ALU.mult)
            veps = tiny.tile([C, 1], F32, tag="veps")
            nc.gpsimd.tensor_tensor(veps[:], mom[:, 1:2], cNe[:], ALU.add)
            vare = tiny.tile([C, 1], F32, tag="vare")
            nc.gpsimd.tensor_tensor(vare[:], veps[:], musq[:], ALU.subtract)
            # rs = gc / sqrt(var+eps) = exp(-0.5*ln(N*(var+eps)) + 0.5*ln N
            #      + ln(gc)); stays in the natural_log_exp ACT table set.
            gc, bc = flags.ln_gc[ln_idx], flags.ln_bc[ln_idx]
            lnv = tiny.tile([C, 1], F32, tag="lnv")
            nc.scalar.activation(lnv[:], vare[:], AF.Ln)
            rs = tiny.tile([C, 1], F32, tag="rs")
            expb = (0.5 * math.log(NEL)
                    + (math.log(gc) if (gc is not None and gc > 0.0) else 0.0))
            nc.scalar.activation(rs[:], lnv[:], AF.Exp, scale=-0.5,
                                 bias=const_ap(expb))
            if gc is not None and gc <= 0.0 and gc != 1.0:
                rs2 = tiny.tile([C, 1], F32, tag="rs2")
                nc.vector.tensor_scalar_mul(rs2[:], rs[:], gc)
                rs = rs2
            # murs = (U/N)*rs so normalize is x*rs - murs (both 4x-capable)
            murs = tiny.tile([C, 1], F32, tag="murs")
            nc.vector.tensor_tensor(murs[:], momn[:, 0:1], rs[:], ALU.mult)
            post = []
            if gc is None:
                post.append(lambda i, o: nc.vector.tensor_tensor(
                    o, i, gb_ap(ln_idx, "g"), ALU.mult))
            if bc is None:
                post.append(lambda i, o: nc.vector.tensor_tensor(
                    o, i, gb_ap(ln_idx, "b"), ALU.add))
            elif bc != 0.0:
                post.append(lambda i, o: nc.vector.tensor_scalar_add(
                    o, i, bc))
            if not post:
                # chunked at NCH so the next conv's first-half matmuls (which
                # read padded cols <= 518) start before the tail is written
                nc.vector.tensor_scalar(dst[:, 0:NCH], src[:, 0:NCH], rs[:],
                                        murs[:], op0=ALU.mult,
                                        op1=ALU.subtract)
                nc.vector.tensor_scalar(dst[:, NCH:T], src[:, NCH:T], rs[:],
                                        murs[:], op0=ALU.mult,
                                        op1=ALU.subtract)
                ctx2.close()
            else:
                ctx2.close()
                mids = [work.tile([C, T], BF16, tag="lnmid", bufs=2),
                        work.tile([C, T], BF16, tag="lnmid2", bufs=2)]
                nc.vector.tensor_scalar(mids[0][:], src, rs[:], murs[:],
                                        op0=ALU.mult, op1=ALU.subtract)
                for i, emit in enumerate(post):
                    last = (i == len(post) - 1)
                    emit(mids[i % 2][:], dst if last else mids[(i + 1) % 2][:])

        # ---- x + pos -> xpad (conv0 input); LN0 -> res ----
        res = resp.tile([C, BPC * T], BF16, tag="res")
        for b in range(BPC):
            stats = tiny.tile([C, 4], F32, tag="stats")
            scr = work.tile([C, T], BF16, tag="sqscr")
            for h in range(2):
                sl = slice(h * H, (h + 1) * H)
                nc.vector.scalar_tensor_tensor(
                    ip(b, h * H, H), xin[:, b * T + h * H:b * T + (h + 1) * H],
                    1.0, pos_sb[:, sl],
                    op0=ALU.mult, op1=ALU.add,
                    accum_out=stats[:, 2 * h:2 * h + 1])
                # h1 square on DVE right after its producer (no engine hop on
                # the stats critical path); h0 square on ACT in parallel
                if h == 0:
                    nc.scalar.activation(scr[:, sl], ip(b, h * H, H),
                                         AF.Square,
                                         accum_out=stats[:, 2 * h + 1:2 * h + 2])
                else:
                    nc.vector.scalar_tensor_tensor(
                        scr[:, sl], ip(b, h * H, H), 1.0, ip(b, h * H, H),
                        op0=ALU.mult, op1=ALU.mult,
                        accum_out=stats[:, 2 * h + 1:2 * h + 2])
            emit_ln_tail(0, b, stats, ip(b), res[:, b * T:(b + 1) * T])

        # ---- conv layers ----
        for li in range(NCONV):
            new_res = resp.tile([C, BPC * T], BF16, tag="res")
            for b in range(BPC):
                stats = tiny.tile([C, 4], F32, tag="stats")
                scr = work.tile([C, T], BF16, tag="sqscr")
                nr = new_res[:, b * T:(b + 1) * T]
                for h in range(2):
                    psc = ps_cv.tile([C, H], F32, tag="ps_cv")
                    for d in range(KW):
                        nc.tensor.matmul(
                            psc[:],
                            w2_sb[:, (li * KW + d) * C:(li * KW + d + 1) * C],
                            xpad[:, b * PADT + h * H + d:
                                 b * PADT + h * H + d + H],
                            start=(d == 0), stop=(d == KW - 1))
                    sl = slice(h * H, (h + 1) * H)
                    if flags.b2_zero:
                        nc.vector.scalar_tensor_tensor(
                            nr[:, sl], psc[:], 0.0,
                            res[:, b * T + h * H:b * T + (h + 1) * H],
                            op0=ALU.max, op1=ALU.add,
                            accum_out=stats[:, 2 * h:2 * h + 1])
                    else:
                        relu_t = work.tile([C, T], BF16, tag="relu_t")
                        nc.scalar.activation(relu_t[:, sl], psc[:],
                                             AF.Relu, bias=b2_sb[:, li:li + 1])
                        nc.vector.scalar_tensor_tensor(
                            nr[:, sl], relu_t[:, sl], 1.0,
                            res[:, b * T + h * H:b * T + (h + 1) * H],
                            op0=ALU.mult, op1=ALU.add,
                            accum_out=stats[:, 2 * h:2 * h + 1])
                    if h == 0:
                        nc.scalar.activation(
                            scr[:, sl], nr[:, sl], AF.Square,
                            accum_out=stats[:, 2 * h + 1:2 * h + 2])
                    else:
                        nc.vector.scalar_tensor_tensor(
                            scr[:, sl], nr[:, sl], 1.0, nr[:, sl],
                            op0=ALU.mult, op1=ALU.mult,
                            accum_out=stats[:, 2 * h + 1:2 * h + 2])
                emit_ln_tail(1 + li, b, stats, nr, ip(b))
            res = new_res

        # ---- attention + LNe + FC, per batch (batches overlap via
        # per-batch PSUM rings) ----
        new_res = resp.tile([C, BPC * T], BF16, tag="res")
        for b in range(BPC):
            pst = f"ps_at{b}"
            stats = tiny.tile([C, 4], F32, tag="stats")
            xa = ip(b)  # [128, 1024] bf16
            # y = M^T x  (scores s^T[tk,tq] = sum_c' x[c',tk] y[c',tq])
            y_sb = work.tile([C, T], BF16, tag="y_sb")
            for h in range(2):
                sl = slice(h * H, (h + 1) * H)
                psy = ps_at.tile([C, H], F32, tag=pst)
                nc.tensor.matmul(psy[:], m_sb, xa[:, sl],
                                 start=True, stop=True)
                nc.vector.tensor_copy(y_sb[:, sl], psy[:])
            # v in [t, d] layout + appended ones column (denominator)
            psv = ps_at.tile([C, H], F32, tag=pst)
            for j in range(8):
                nc.tensor.matmul(psv[:, j * DK:(j + 1) * DK],
                                 xa[:, j * C:(j + 1) * C], wv_sb,
                                 start=True, stop=True)
            vt = work.tile([C, 8, DK + 1], BF16, tag="vt")
            nc.vector.memset(vt[:, :, DK:DK + 1], 1.0)
            nc.vector.tensor_copy(
                vt[:, :, 0:DK],
                psv[:].rearrange("p (j k) -> p j k", k=DK))
            # scores (transposed: [tk, tq]) + exp, per (key chunk, half)
            eT = work.tile([C, 8 * T], BF16, tag="eT", bufs=2)
            for j in range(8):
                for h in range(2):
                    pss = ps_at.tile([C, H], F32, tag=pst)
                    nc.tensor.matmul(pss[:],
                                     xa[:, j * C:(j + 1) * C],
                                     y_sb[:, h * H:(h + 1) * H],
                                     start=True, stop=True)
                    esl = slice(j * T + h * H, j * T + (h + 1) * H)
                    if flags.mask_ones:
                        nc.scalar.activation(eT[:, esl], pss[:], AF.Exp)
                    else:
                        nc.scalar.activation(
                            eT[:, esl], pss[:], AF.Exp,
                            bias=mb_sb[:, b * 8 + j:b * 8 + j + 1])
            # attention-weighted values + denominator (appended ones column)
            psa = ps_av.tile([DK + 1, T], F32, tag="ps_av")
            for h in range(2):
                for j in range(8):
                    nc.tensor.matmul(
                        psa[:, h * H:(h + 1) * H], vt[:, j, :],
                        eT[:, j * T + h * H: j * T + h * H + H],
                        start=(j == 0), stop=(j == 7))
            # Per-half tail: reciprocal of the denominator row on DVE,
            # broadcast to partitions via a PE ones-matmul (PE and ACT are
            # idle in the tail; DVE is the contended engine), av copy on ACT,
            # normalize+cast on DVE, Wo matmul.
            rr = tiny.tile([1, T], F32R, tag="rr", bufs=2)
            av = work.tile([DK, T], BF16, tag="av")
            rrb = work.tile([DK, T], F32, tag="rrb", bufs=2)
            avn = work.tile([DK, T], BF16, tag="avn")
            nr = new_res[:, b * T:(b + 1) * T]
            scr = work.tile([C, T], BF16, tag="sqscr")
            # b0's tail overlaps b1's softmax exps (ACT saturated), so it
            # normalizes via Pool partition_broadcast + DVE; the last batch's
            # tail has ACT and PE idle, so it uses an ACT av-copy and a PE
            # ones-matmul broadcast instead.
            last = (b == BPC - 1)
            for h in range(2):
                sl = slice(h * H, (h + 1) * H)
                with nc.allow_low_precision("softmax denom reciprocal; "
                                             "f32r tag only changes mm mode"):
                    if flags.mask_ones:
                        nc.vector.reciprocal(rr[:, sl], psa[DK:DK + 1, sl])
                    else:
                        # guard fully-masked queries (denom==0 -> finite rr;
                        # av rows are 0 so the product stays 0)
                        nc.vector.tensor_scalar_add(rr[:, sl],
                                                    psa[DK:DK + 1, sl], 1e-30)
                        nc.vector.reciprocal(rr[:, sl], rr[:, sl])
                if last:
                    nc.scalar.copy(av[:, sl], psa[0:DK, sl])
                    psr = ps_at.tile([C, H], F32, tag=pst)
                    nc.tensor.matmul(psr[0:DK, :], ones_row_r[:], rr[:, sl],
                                     start=True, stop=True)
                    nc.vector.tensor_tensor(avn[:, sl], av[:, sl],
                                            psr[0:DK, :], ALU.mult)
                else:
                    nc.gpsimd.partition_broadcast(
                        rrb[:, sl], rr[:, sl].bitcast(F32), channels=DK)
                    nc.vector.tensor_tensor(avn[:, sl], psa[0:DK, sl],
                                            rrb[:, sl], ALU.mult)
                pso = ps_at.tile([C, H], F32, tag=pst)
                nc.tensor.matmul(pso[:], wo_sb, avn[:, sl],
                                 start=True, stop=True)
                rsl = slice(b * T + h * H, b * T + (h + 1) * H)
                if flags.mask_ones:
                    nc.vector.scalar_tensor_tensor(
                        nr[:, sl], pso[:], 1.0, res[:, rsl],
                        op0=ALU.mult, op1=ALU.add,
                        accum_out=stats[:, 2 * h:2 * h + 1])
                else:
                    # query-mask rows broadcast across partitions (Pool),
                    # applied before the residual add
                    qmb = work.tile([C, T], F32, tag="qmb")
                    att = work.tile([C, T], BF16, tag="att_m")
                    nc.gpsimd.partition_broadcast(
                        qmb[:, sl],
                        qm_sb[:, b * T + h * H:b * T + (h + 1) * H],
                        channels=C)
                    nc.vector.tensor_tensor(att[:, sl], pso[:], qmb[:, sl],
                                            ALU.mult)
                    nc.vector.scalar_tensor_tensor(
                        nr[:, sl], att[:, sl], 1.0, res[:, rsl],
                        op0=ALU.mult, op1=ALU.add,
                        accum_out=stats[:, 2 * h:2 * h + 1])
                if last:
                    nc.scalar.activation(
                        scr[:, sl], nr[:, sl], AF.Square,
                        accum_out=stats[:, 2 * h + 1:2 * h + 2])
                else:
                    nc.vector.scalar_tensor_tensor(
                        scr[:, sl], nr[:, sl], 1.0, nr[:, sl],
                        op0=ALU.mult, op1=ALU.mult,
                        accum_out=stats[:, 2 * h + 1:2 * h + 2])
            emit_ln_tail(NLN - 1, b, stats, nr, ip(b))
            # ---- final FC + relu + residual (same-batch PSUM ring) ----
            for h in range(2):
                psf = ps_at.tile([C, H], F32, tag=pst)
                nc.tensor.matmul(psf[:], fcw_sb, ip(b, h * H, H),
                                 start=True, stop=True)
                sl = slice(h * H, (h + 1) * H)
                ob = out_sb[:, b * T + h * H:b * T + (h + 1) * H]
                rb_ = nr[:, sl]
                if flags.fcb_zero:
                    nc.vector.scalar_tensor_tensor(
                        ob, psf[:], 0.0, rb_, op0=ALU.max, op1=ALU.add)
                else:
                    relu_t = work.tile([C, T], BF16, tag="relu_t")
                    nc.scalar.activation(relu_t[:, sl], psf[:], AF.Relu,
                                         bias=fcb_sb[:])
                    nc.vector.tensor_tensor(ob, relu_t[:, sl], rb_, ALU.add)
                (nc.sync if h == 0 else nc.scalar).dma_start(
                    out_d[b][:, sl], ob)

    nc.compile()
    return nc


_CACHE: dict = {}
LAST_RUN: dict = {}   # exposed for test harnesses (nc, in_maps)


def kernel(x, mask, dw_w, dw_b, pw_w, pw_b, norm0_g, norm0_b,
           norms_g, norms_b, norme_g, norme_b,
           Wq, Wk, Wv, Wo, fc_w, fc_b):
    x = np.asarray(x, dtype=np.float32)
    mask = np.asarray(mask, dtype=np.float32)

    # ---- host-side constant folding ----
    w2 = np.empty((C, NCONV, KW, C), dtype=np.float32)
    for i in range(NCONV):
        pwT = np.asarray(pw_w[i], np.float32).T          # [c, o]
        for d in range(KW):
            w2[:, i, d, :] = pwT * np.asarray(dw_w[i][:, d],
                                              np.float32)[:, None]
    w2 = w2.reshape(C, NCONV * KW * C)
    b2 = np.stack([np.asarray(pw_w[i], np.float32)
                   @ np.asarray(dw_b[i], np.float32)
                   + np.asarray(pw_b[i], np.float32)
                   for i in range(NCONV)], axis=1)        # [C, NCONV]
    mqk = (np.asarray(Wq, np.float32) @ np.asarray(Wk, np.float32).T
           / math.sqrt(DK))                               # [C, C]
    wo = np.asarray(Wo, np.float32)
    wo_eff = np.ascontiguousarray(wo[:DK] + wo[DK:])      # [DK, C]
    fcw = np.ascontiguousarray(np.asarray(fc_w, np.float32).T)  # [c, o]
    fcb = np.asarray(fc_b, np.float32).reshape(C, 1)
    pos = _pos_encoding()

    gs = [norm0_g] + [norms_g[i] for i in range(NCONV)] + [norme_g]
    bs = [norm0_b] + [norms_b[i] for i in range(NCONV)] + [norme_b]
    ln_gc = [_uniform_val(np.asarray(g, np.float32)) for g in gs]
    ln_bc = [_uniform_val(np.asarray(bb, np.float32)) for bb in bs]
    flags = _Flags(ln_gc, ln_bc,
                   b2_zero=not b2.any(),
                   fcb_zero=not fcb.any(),
                   mask_ones=bool(np.all(mask == 1.0)))

    key = flags.key()
    if key not in _CACHE:
        _CACHE[key] = _build(flags)
    nc = _CACHE[key]

    wcat = np.zeros((C, 3 * C + DK), np.float32)
    wcat[:, 0:C] = mqk
    wcat[:, C:C + DK] = np.asarray(Wv, np.float32)
    wcat[:, C + DK:2 * C + DK] = fcw
    wcat[0:DK, 2 * C + DK:3 * C + DK] = wo_eff
    base = {"pos": _bf(pos), "w2": _bf(w2), "wcat": _bf(wcat)}
    if not flags.b2_zero:
        base["b2"] = np.ascontiguousarray(b2)
    if not flags.fcb_zero:
        base["fcb"] = np.ascontiguousarray(fcb)
    gb_entries = ([(l, "g") for l in range(NLN) if flags.ln_gc[l] is None]
                  + [(l, "b") for l in range(NLN) if flags.ln_bc[l] is None])
    if gb_entries:
        gb = np.empty((C, len(gb_entries) * T), np.float32)
        for i, (l, kind) in enumerate(gb_entries):
            src = gs[l] if kind == "g" else bs[l]
            gb[:, i * T:(i + 1) * T] = np.asarray(src, np.float32)
        base["gb"] = _bf(gb)

    in_maps = []
    for c in range(NCORES):
        m = dict(base)
        xc = x[c * BPC:(c + 1) * BPC]                     # [BPC, C, T]
        m["x"] = _bf(xc.transpose(1, 0, 2).reshape(C, BPC * T))
        if not flags.mask_ones:
            msk = mask[c * BPC:(c + 1) * BPC]             # [BPC, T]
            mb = np.where(msk == 0.0, np.float32(-1e9), np.float32(0.0))
            # [128, BPC*8]: column (b*8+j) = bias for key chunk j of batch b
            m["mb"] = np.ascontiguousarray(
                mb.reshape(BPC, 8, C).transpose(2, 0, 1).reshape(C, BPC * 8))
            m["qm"] = np.ascontiguousarray(msk)
        in_maps.append(m)

    LAST_RUN["nc"] = nc
    LAST_RUN["in_maps"] = in_maps

    res = run_bass_kernel_spmd(nc, in_maps, list(range(NCORES)))
    out = np.concatenate([np.asarray(r["out"]) for r in res.results], axis=0)
    return out.astype(np.float32)
